# revision 27
# baseline (speedup 1.0000x reference)
"""Trainium2 Bass kernel for nn_AttnGCN (2-layer GATv2 + BN + dropout + FC).

Sharding: nodes are partitioned across 8 NeuronCores (graph parallel).  Each
core owns a contiguous range of 6250 destination nodes (padded to 6272 =
49*128).  Edges are bucketed by destination tile on the host (index-only
preprocessing), each tile's edge list padded to whole 128-edge blocks.  Layer-1
runs per-core on the edge shard; BN statistics are combined with a tiny
AllReduce; the activated layer-1 features are AllGathered (bf16) so every core
can gather arbitrary source rows for layer-2; layer-2 + FC produce the owned
output shard, which the host concatenates.

All numeric work (matmuls, softmax, scatter/gather, BN, masking) happens on
device.  Host does only index bucketing, parameter layout, and output
reassembly.  Dropout masks are the fixed jax PRNG streams of the reference
(input-independent constants), computed once on host CPU.
"""

import os
import sys
import types
import numpy as np
import ml_dtypes

import concourse.bacc as bacc
import concourse.bass as bass
import concourse.mybir as mybir
import concourse.tile as tile
from concourse.bass_utils import run_bass_kernel_spmd
from concourse.masks import make_identity

P = 128
NCORES = 8
N = 50000
E = 400000
NCF = 26          # input/output feature dim
H = 2
CH = 128
HC = 256
NOWN = N // NCORES            # 6250 owned nodes per core
NT = (NOWN + P - 1) // P      # 49 node tiles per core
NPC = NT * P                  # 6272 padded nodes per core
NPAD = NCORES * NPC           # 50176
NREAL_LAST = NOWN - (NT - 1) * P   # 106 real nodes in last tile
DP_SCALE = 1.25               # 1/(1-0.2)
KA = NCF + 1                  # 27: x features + edge weight (for w*We fold)

FP32 = mybir.dt.float32
F32R = mybir.dt.float32r
BF16 = mybir.dt.bfloat16
I32 = mybir.dt.int32
AF = mybir.ActivationFunctionType
OP = mybir.AluOpType
RG = [list(range(NCORES))]
BF = ml_dtypes.bfloat16

_PROGRAM_CACHE = {}
_MASK_CACHE = {}

_MASK_SCRIPT = r"""
import os, sys
for _p in reversed(os.environ.get("NIX_PYTHONPATH", "").split(os.pathsep)):
    if _p and _p not in sys.path:
        sys.path.insert(0, _p)
import numpy as np
import jax
m1 = np.asarray(jax.random.bernoulli(jax.random.key(1), 0.8, (%d, %d)),
                dtype=np.float32)
m2 = np.asarray(jax.random.bernoulli(jax.random.key(2), 0.8, (%d, %d)),
                dtype=np.float32)
np.savez(sys.argv[1], m1=m1, m2=m2)
"""


def _dropout_masks():
    """Reference dropout masks: fixed jax PRNG streams, computed with plain
    CPU jax (subprocess) so the bit stream matches a stock jax environment."""
    if "m" not in _MASK_CACHE:
        import subprocess
        import tempfile
        env = dict(os.environ)
        env["JAX_PLATFORMS"] = "cpu"
        env.pop("XLA_FLAGS", None)
        env.pop("TRN_TERMINAL_POOL_IPS", None)
        with tempfile.TemporaryDirectory() as td:
            fn = os.path.join(td, "masks.npz")
            script = _MASK_SCRIPT % (N, HC, N, HC)
            r = subprocess.run([sys.executable, "-c", script, fn], env=env,
                               capture_output=True, text=True)
            if r.returncode != 0:
                raise RuntimeError("mask subprocess failed: " + r.stderr[-2000:])
            d = np.load(fn)
            _MASK_CACHE["m"] = (d["m1"] * DP_SCALE, d["m2"] * DP_SCALE)
    return _MASK_CACHE["m"]


# ----------------------------------------------------------------------------
# host-side index preprocessing (sharding)
# ----------------------------------------------------------------------------

def _host_prep(x_input, edge_weight, params, edge_index):
    src = np.asarray(edge_index[0], dtype=np.int64).astype(np.int32)
    dst = np.asarray(edge_index[1], dtype=np.int64).astype(np.int32)
    w = np.asarray(edge_weight, dtype=np.float32).reshape(-1)
    x = np.asarray(x_input, dtype=np.float32)

    r_arr = dst // NOWN
    nloc = dst - r_arr * NOWN
    t_arr = nloc >> 7
    dstloc = (nloc & 127).astype(np.int32)

    key = r_arr * NT + t_arr
    order = np.argsort(key, kind="stable")
    cnt = np.bincount(key, minlength=NCORES * NT).reshape(NCORES, NT)
    bounds = np.concatenate([[0], np.cumsum(cnt.reshape(-1))]).astype(np.int64)

    # ----- layer 1 blocks -----
    NBT1 = np.maximum(1, -(-cnt.max(0) // P)).astype(np.int64)
    off1 = np.concatenate([[0], np.cumsum(NBT1[:-1])]).astype(np.int64)
    LB1 = int(NBT1.sum())
    LE1 = LB1 * P
    edges1 = np.zeros((NCORES, LE1, 3), np.int32)
    edges1[:, :, 1] = 255

    for r in range(NCORES):
        for t in range(NT):
            k = r * NT + t
            c = int(cnt[r, t])
            if c:
                sl = order[bounds[k]:bounds[k] + c]
                base = int(off1[t]) * P
                edges1[r, base:base + c, 0] = src[sl]
                edges1[r, base:base + c, 1] = dstloc[sl]
                edges1[r, base:base + c, 2] = w[sl].view(np.int32)

    # ----- layer 2 blocks (real edges + self loops) -----
    nreal = np.full(NT, P, np.int64)
    nreal[NT - 1] = NREAL_LAST
    cnt2 = cnt + nreal[None, :]
    NBT2 = np.maximum(1, -(-cnt2.max(0) // P)).astype(np.int64)
    off2 = np.concatenate([[0], np.cumsum(NBT2[:-1])]).astype(np.int64)
    LB2 = int(NBT2.sum())
    LE2 = LB2 * P
    LE2P = LE2 + P
    g2src = (src // NOWN) * NPC + src % NOWN   # padded-global source ids

    edges2 = np.zeros((NCORES, LE2, 2), np.int32)
    edges2[:, :, 1] = 255
    w2x = np.zeros((NCORES, LE2P), np.float32)
    slots2 = np.zeros((NCORES, NT * P), np.int32)

    for r in range(NCORES):
        for t in range(NT):
            k = r * NT + t
            c = int(cnt[r, t])
            sl = order[bounds[k]:bounds[k] + c]
            base = int(off2[t]) * P
            edges2[r, base:base + c, 0] = g2src[sl]
            edges2[r, base:base + c, 1] = dstloc[sl]
            w2x[r, base:base + c] = w[sl]
            nr = int(nreal[t])
            pos = base + c
            edges2[r, pos:pos + nr, 0] = r * NPC + t * P + np.arange(nr)
            edges2[r, pos:pos + nr, 1] = np.arange(nr)
            slots2[r, t * P:t * P + nr] = pos + np.arange(nr)
            slots2[r, t * P + nr:(t + 1) * P] = LE2 + np.arange(nr, P)

    # ----- degrees / masks / params -----
    deg = np.bincount(dst, minlength=N).astype(np.float32)
    invdeg_full = 1.0 / np.maximum(deg, 1.0)
    invdeg = np.ones((NCORES, NPC), np.float32)
    for r in range(NCORES):
        invdeg[r, :NOWN] = invdeg_full[r * NOWN:(r + 1) * NOWN]

    m1, m2 = _dropout_masks()

    def shard_rows(a, dtype=np.float32):
        out = np.zeros((NCORES, NPC) + a.shape[1:], dtype)
        for r in range(NCORES):
            out[r, :NOWN] = a[r * NOWN:(r + 1) * NOWN]
        return out

    mask1 = shard_rows(m1)
    mask2 = shard_rows(m2)
    x_own = shard_rows(x, BF)

    p = {k: np.asarray(v, dtype=np.float32) for k, v in params.items()}
    wfc_pk = np.zeros((P, 52), BF)
    wfc_pk[:, :26] = p["Wfc"][:128].astype(BF)
    wfc_pk[:, 26:] = p["Wfc"][128:].astype(BF)
    common = {
        "xtb": x.T.astype(BF).copy(),
        "Wl1": p["Wl1"].astype(BF),
        "we1row": p["We1"].reshape(1, HC).astype(BF),
        "Wr1": p["Wr1"].astype(BF),
        "att1r": np.tile(p["att1"].reshape(1, HC), (P, 1)).astype(BF),
        "Wl2": p["Wl2"].astype(BF), "Wr2": p["Wr2"].astype(BF),
        "we2row": p["We2"].reshape(1, HC).astype(BF),
        "we1r": np.tile(p["We1"].reshape(1, HC), (P, 1)),
        "we2r": np.tile(p["We2"].reshape(1, HC), (P, 1)),
        "att2r": np.tile(p["att2"].reshape(1, HC), (P, 1)).astype(BF),
        "wfc": wfc_pk,
        "bfcr": np.tile(p["bfc"].reshape(1, NCF), (P, 1)),
        "g1row": p["g1"].reshape(1, HC), "be1row": p["be1"].reshape(1, HC),
        "g2row": p["g2"].reshape(1, HC), "be2row": p["be2"].reshape(1, HC),
    }
    in_maps = []
    for r in range(NCORES):
        m = dict(common)
        m["edges1"] = edges1[r]
        m["w1c"] = edges1[r][:, 2].view(np.float32).copy()[:, None]
        m["edges2"] = edges2[r]
        m["w2x"] = w2x[r][:, None]
        m["slots2"] = slots2[r][:, None]
        m["invdeg"] = invdeg[r][:, None]
        m["mask1"] = mask1[r]
        m["mask2"] = mask2[r]
        m["x_own"] = x_own[r]
        in_maps.append(m)

    meta = dict(NBT1=tuple(int(v) for v in NBT1), off1=tuple(int(v) for v in off1),
                NBT2=tuple(int(v) for v in NBT2), off2=tuple(int(v) for v in off2),
                LE1=LE1, LE2=LE2, LE2P=LE2P)
    return in_maps, meta


# ----------------------------------------------------------------------------
# device program
# ----------------------------------------------------------------------------

def _build_program(meta):
    NBT1, off1 = meta["NBT1"], meta["off1"]
    NBT2, off2 = meta["NBT2"], meta["off2"]
    LE1, LE2, LE2P = meta["LE1"], meta["LE2"], meta["LE2P"]

    nc = bacc.Bacc("TRN2", target_bir_lowering=False)

    # ---- I/O ----
    xtbT = nc.dram_tensor("xtb", (NCF, N), BF16, kind="ExternalInput")
    e1T = nc.dram_tensor("edges1", (LE1, 3), I32, kind="ExternalInput")
    e2T = nc.dram_tensor("edges2", (LE2, 2), I32, kind="ExternalInput")
    w2xT = nc.dram_tensor("w2x", (LE2P, 1), FP32, kind="ExternalInput")
    slotsT = nc.dram_tensor("slots2", (NT * P, 1), I32, kind="ExternalInput")
    invdT = nc.dram_tensor("invdeg", (NPC, 1), FP32, kind="ExternalInput")
    mask1T = nc.dram_tensor("mask1", (NPC, HC), FP32, kind="ExternalInput")
    mask2T = nc.dram_tensor("mask2", (NPC, HC), FP32, kind="ExternalInput")
    xownT = nc.dram_tensor("x_own", (NPC, NCF), BF16, kind="ExternalInput")
    Wl1T = nc.dram_tensor("Wl1", (NCF, HC), BF16, kind="ExternalInput")
    we1rowT = nc.dram_tensor("we1row", (1, HC), BF16, kind="ExternalInput")
    w1cT = nc.dram_tensor("w1c", (LE1, 1), FP32, kind="ExternalInput")
    Wr1T = nc.dram_tensor("Wr1", (NCF, HC), BF16, kind="ExternalInput")
    att1rT = nc.dram_tensor("att1r", (P, HC), BF16, kind="ExternalInput")
    Wl2T = nc.dram_tensor("Wl2", (HC, HC), BF16, kind="ExternalInput")
    Wr2T = nc.dram_tensor("Wr2", (HC, HC), BF16, kind="ExternalInput")
    we2rowT = nc.dram_tensor("we2row", (1, HC), BF16, kind="ExternalInput")
    we1rT = nc.dram_tensor("we1r", (P, HC), FP32, kind="ExternalInput")
    we2rT = nc.dram_tensor("we2r", (P, HC), FP32, kind="ExternalInput")
    att2rT = nc.dram_tensor("att2r", (P, HC), BF16, kind="ExternalInput")
    wfcT = nc.dram_tensor("wfc", (P, 52), BF16, kind="ExternalInput")
    bfcrT = nc.dram_tensor("bfcr", (P, NCF), FP32, kind="ExternalInput")
    g1rT = nc.dram_tensor("g1row", (1, HC), FP32, kind="ExternalInput")
    be1rT = nc.dram_tensor("be1row", (1, HC), FP32, kind="ExternalInput")
    g2rT = nc.dram_tensor("g2row", (1, HC), FP32, kind="ExternalInput")
    be2rT = nc.dram_tensor("be2row", (1, HC), FP32, kind="ExternalInput")
    outT = nc.dram_tensor("out", (NPC, NCF), FP32, kind="ExternalOutput")

    # ---- internal DRAM ----
    dbg = bool(os.environ.get("KERNEL_DBG"))
    dbgkind = {"kind": "ExternalOutput"} if dbg else {}
    NXL1 = ((N + P - 1) // P) * P
    hpre1 = nc.dram_tensor("hpre1", (NPC, HC), FP32, **dbgkind)
    h1dbg = (nc.dram_tensor("h1dbg", (NPC, HC), FP32, kind="ExternalOutput")
             if dbg else None)
    xl1 = nc.dram_tensor("xl1", (NXL1, HC), BF16)
    xl2loc = nc.dram_tensor("xl2loc", (NPC, HC), BF16)
    xl2full = nc.dram_tensor("xl2full", (NPAD, HC), BF16, addr_space="Shared")
    hpre2 = nc.dram_tensor("hpre2", (NPC, HC), FP32, **dbgkind)
    w2i = nc.dram_tensor("w2i", (LE2P, 1), FP32, **dbgkind)
    st1i = nc.dram_tensor("st1i", (1, 2 * HC), FP32)
    st1o = nc.dram_tensor("st1o", (1, 2 * HC), FP32, addr_space="Shared")
    st2i = nc.dram_tensor("st2i", (1, 2 * HC), FP32)
    st2o = nc.dram_tensor("st2o", (1, 2 * HC), FP32, addr_space="Shared")

    with tile.TileContext(nc) as tc:
        with tc.tile_pool(name="cst", bufs=1) as cst, \
             tc.tile_pool(name="sb", bufs=4) as sb, \
             tc.tile_pool(name="mm", bufs=3, space="PSUM") as mmp, \
             tc.tile_pool(name="tr", bufs=3, space="PSUM") as trp, \
             tc.tile_pool(name="accp", bufs=2, space="PSUM") as accp:

            # ---------------- constants ----------------
            iota_i = cst.tile([P, P], I32, tag="iota_i")
            nc.gpsimd.iota(iota_i[:], pattern=[[1, P]], base=0,
                           channel_multiplier=0)
            iota_f = cst.tile([P, P], FP32, tag="iota_f")
            nc.vector.tensor_copy(iota_f[:], iota_i[:])
            ident32 = cst.tile([P, P], FP32, tag="ident32")
            make_identity(nc, ident32[:])
            identb = cst.tile([P, P], BF16, tag="identb")
            nc.vector.tensor_copy(identb[:], ident32[:])

            def load_const(name, dram, shape, dtype=FP32):
                t = cst.tile(shape, dtype, tag=name)
                nc.sync.dma_start(out=t[:], in_=dram[:, :])
                return t

            Wl1s = load_const("Wl1s", Wl1T, [NCF, HC], BF16)
            we1rs = load_const("we1rs", we1rowT, [1, HC], BF16)
            Wr1s = load_const("Wr1s", Wr1T, [NCF, HC], BF16)
            att1s = load_const("att1s", att1rT, [P, HC], BF16)
            we2rs = load_const("we2rs", we2rowT, [1, HC], BF16)
            we1rep = load_const("we1rep", we1rT, [P, HC])
            we2rep = load_const("we2rep", we2rT, [P, HC])
            att2s = load_const("att2s", att2rT, [P, HC], BF16)
            wfcs = load_const("wfcs", wfcT, [P, 52], BF16)
            bfcs = load_const("bfcs", bfcrT, [P, NCF])
            g1s = load_const("g1s", g1rT, [1, HC])
            be1s = load_const("be1s", be1rT, [1, HC])
            g2s = load_const("g2s", g2rT, [1, HC])
            be2s = load_const("be2s", be2rT, [1, HC])
            Wl2s = cst.tile([P, 2 * HC], BF16, tag="Wl2s")
            Wr2s = cst.tile([P, 2 * HC], BF16, tag="Wr2s")
            for kk in range(2):
                nc.sync.dma_start(out=Wl2s[:, kk * HC:(kk + 1) * HC],
                                  in_=Wl2T[kk * P:(kk + 1) * P, :])
                nc.sync.dma_start(out=Wr2s[:, kk * HC:(kk + 1) * HC],
                                  in_=Wr2T[kk * P:(kk + 1) * P, :])
            ones_col = cst.tile([P, 1], FP32, tag="ones_col")
            nc.vector.memset(ones_col[:], 1.0)
            ones_row = cst.tile([1, P], FP32, tag="ones_row")
            nc.vector.memset(ones_row[:], 1.0)

            st1sb = cst.tile([1, 2 * HC], FP32, tag="st1sb")
            nc.vector.memset(st1sb[:], 0.0)
            st2sb = cst.tile([1, 2 * HC], FP32, tag="st2sb")
            nc.vector.memset(st2sb[:], 0.0)

            # copy host edge weights for layer 2 (self-loop slots get filled
            # by the device during layer-1 finalize)
            nrows = LE2P // P
            for c0 in range(0, nrows, P):
                cn = min(P, nrows - c0)
                w2cp = sb.tile([P, P], FP32, tag="w2cp")
                nc.sync.dma_start(
                    out=w2cp[:cn, :],
                    in_=w2xT[:, 0].rearrange("(a b) -> a b", b=P)[c0:c0 + cn, :])
                nc.sync.dma_start(
                    out=w2i[:, 0].rearrange("(a b) -> a b", b=P)[c0:c0 + cn, :],
                    in_=w2cp[:cn, :])

            # ---------- XL1 = x @ Wl1 for every node (replicated) ----------
            GRP = 4
            nchunks = (N + P - 1) // P          # 391
            for j0 in range(0, nchunks, GRP):
                jn = min(GRP, nchunks - j0)
                c0 = j0 * P
                cn = min(GRP * P, N - c0)
                xtb_sb = sb.tile([NCF, GRP * P], BF16, tag="xtb_sb")
                nc.sync.dma_start(out=xtb_sb[:, 0:cn],
                                  in_=xtbT[:, c0:c0 + cn])
                xl_sb = sb.tile([P, GRP * HC], BF16, tag="xl_sb")
                for j in range(jn):
                    n0 = (j0 + j) * P
                    nn = min(P, N - n0)
                    xl_ps = mmp.tile([P, HC], FP32, tag="mm")
                    nc.tensor.matmul(out=xl_ps[:nn, :],
                                     lhsT=xtb_sb[:, j * P:j * P + nn],
                                     rhs=Wl1s[:], start=True, stop=True)
                    nc.vector.tensor_copy(xl_sb[:nn, j * HC:(j + 1) * HC],
                                          xl_ps[:nn, :])
                full = jn * P if (j0 + jn) * P <= N else None
                if full:
                    nc.sync.dma_start(
                        out=xl1[c0:c0 + jn * P, :].rearrange(
                            "(j p) c -> p j c", p=P),
                        in_=xl_sb[:].rearrange(
                            "p (j c) -> p j c", c=HC)[:, 0:jn, :])
                else:
                    for j in range(jn):
                        n0 = (j0 + j) * P
                        nn = min(P, N - n0)
                        nc.sync.dma_start(out=xl1[n0:n0 + nn, :],
                                          in_=xl_sb[:nn, j * HC:(j + 1) * HC])

            # ================= generic GATv2 edge layer =================
            # Per block:  m = G + XR[dst] + w*We  accumulated in ONE psum
            # group on PE.  The destination-scatter aggregates p*m; the
            # XR/We parts are removed per-tile:
            #   num = S - xr (x) den - We (x) sum(p*w),   h = num / den.
            def edge_layer(layer):
                if layer == 1:
                    NBT, off, eT = NBT1, off1, e1T
                    atts = att1s
                    hpre_dram = hpre1
                    stsb = st1sb
                    ew = 3   # ints per edge record
                    wers, werep = we1rs, we1rep
                    vw = 262
                else:
                    NBT, off, eT = NBT2, off2, e2T
                    atts = att2s
                    hpre_dram = hpre2
                    stsb = st2sb
                    ew = 2
                    wers, werep = we2rs, we2rep
                    vw = 260

                for t in range(NT):
                    nb = NBT[t]
                    # ---- XR tile for the 128 owned nodes ----
                    if layer == 1:
                        xo = sb.tile([P, NCF], BF16, tag="xo")
                        nc.sync.dma_start(out=xo[:],
                                          in_=xownT[t * P:(t + 1) * P, :])
                        xoT_ps = trp.tile([P, P], BF16, tag="tr")
                        nc.tensor.transpose(out=xoT_ps[:NCF, :], in_=xo[:],
                                            identity=identb[:])
                        xoTs = sb.tile([NCF, P], BF16, tag="xoTs")
                        nc.vector.tensor_copy(xoTs[:], xoT_ps[:NCF, :])
                        xr_ps = mmp.tile([P, HC], FP32, tag="mm")
                        nc.tensor.matmul(out=xr_ps[:], lhsT=xoTs[:],
                                         rhs=Wr1s[:], start=True, stop=True)
                        xr_sb = sb.tile([P, HC], BF16, tag="xr_sb")
                        nc.scalar.activation(xr_sb[:], xr_ps[:], AF.Copy)
                    else:
                        xr_sb = xr2_all[t]

                    # ---- whole tile's edge records in one DMA ----
                    o0 = off[t] * P
                    ebt = sb.tile([P, nb * ew], I32, tag="ebt")
                    nc.sync.dma_start(
                        out=ebt[:].rearrange("p (b c) -> p b c", c=ew),
                        in_=eT[o0:o0 + nb * P, :].rearrange(
                            "(b p) c -> p b c", p=P))
                    wsrc = w1cT if layer == 1 else w2i
                    wrf = sb.tile([1, nb * P], FP32, tag="wrf")
                    nc.sync.dma_start(
                        out=wrf[:],
                        in_=wsrc[o0:o0 + nb * P, :].rearrange("a one -> one a"))
                    wrowt = sb.tile([1, nb * P], BF16, tag="wrowt")
                    nc.vector.tensor_copy(wrowt[:], wrf[:])
                    # per-partition w and float dst for the whole tile
                    wpp = sb.tile([P, nb], FP32, tag="wpp")
                    nc.sync.dma_start(
                        out=wpp[:],
                        in_=wsrc[:, 0].rearrange(
                            "(a p) -> p a", p=P)[:, off[t]:off[t] + nb])
                    d_ft = sb.tile([P, nb], FP32, tag="d_ft")
                    nc.vector.tensor_copy(
                        d_ft[:], ebt[:].rearrange("p (b c) -> p b c", c=ew)[:, :, 1])

                    acc = accp.tile([P, 262], FP32, tag="acc")
                    for b in range(nb):
                        sidx = ebt[:, b * ew:b * ew + 1]

                        # ---- one-hot by local destination ----
                        oh = sb.tile([P, P], BF16, tag="oh")
                        nc.vector.tensor_scalar(
                            out=oh[:], in0=iota_f[:], scalar1=d_ft[:, b:b + 1],
                            scalar2=None, op0=OP.is_equal)
                        ohT_ps = trp.tile([P, P], BF16, tag="tr")
                        nc.tensor.transpose(out=ohT_ps[:], in_=oh[:],
                                            identity=identb[:])
                        ohTs = sb.tile([P, P], BF16, tag="ohTs")
                        nc.scalar.activation(ohTs[:], ohT_ps[:], AF.Copy)

                        # ---- gather pre-transformed source rows ----
                        gsrc = xl1 if layer == 1 else xl2full
                        grow = sb.tile([P, HC], BF16, tag="grow")
                        nc.gpsimd.indirect_dma_start(
                            out=grow[:], out_offset=None, in_=gsrc[:, :],
                            in_offset=bass.IndirectOffsetOnAxis(
                                ap=sidx, axis=0))
                        # ---- XR[dst] + w*We in one psum group ----
                        m_ps = mmp.tile([P, HC], FP32, tag="mm")
                        nc.tensor.matmul(out=m_ps[:], lhsT=ohTs[:],
                                         rhs=xr_sb[:], start=True, stop=False)
                        nc.tensor.matmul(out=m_ps[:],
                                         lhsT=wrowt[:, b * P:(b + 1) * P],
                                         rhs=wers[:], start=False, stop=True)
                        m_sb = sb.tile([P, HC], BF16, tag="m_sb")
                        nc.vector.tensor_add(m_sb[:], grow[:], m_ps[:])

                        # ---- attention logits from Prelu(m) ----
                        lrm = sb.tile([P, HC], BF16, tag="lrm")
                        nc.scalar.activation(lrm[:], m_sb[:], AF.Prelu,
                                             alpha=0.2)
                        junk = sb.tile([P, P], BF16, tag="junk")
                        alpha = sb.tile([P, 2], FP32, tag="alpha")
                        for hh in range(2):
                            nc.vector.scalar_tensor_tensor(
                                out=junk[:],
                                in0=lrm[:, hh * CH:(hh + 1) * CH], scalar=1.0,
                                in1=atts[:, hh * CH:(hh + 1) * CH],
                                op0=OP.mult, op1=OP.mult,
                                accum_out=alpha[:, hh:hh + 1])
                        pexp = sb.tile([P, 2], FP32, tag="pexp")
                        nc.scalar.activation(pexp[:], alpha[:], AF.Exp)

                        # ---- V = [p*m | p | p*w | (w,w)]; scatter by dst ----
                        v = sb.tile([P, 262], BF16, tag="v")
                        nc.vector.tensor_scalar_mul(v[:, 0:CH], m_sb[:, 0:CH],
                                                    pexp[:, 0:1])
                        nc.vector.tensor_scalar_mul(v[:, CH:HC], m_sb[:, CH:HC],
                                                    pexp[:, 1:2])
                        nc.vector.tensor_copy(v[:, HC:HC + 2], pexp[:])
                        nc.vector.tensor_scalar_mul(v[:, HC + 2:HC + 4],
                                                    pexp[:], wpp[:, b:b + 1])
                        if layer == 1:
                            nc.vector.tensor_copy(
                                v[:, HC + 4:HC + 6],
                                wpp[:, b:b + 1].to_broadcast([P, 2]))
                        nc.tensor.matmul(out=acc[:, 0:vw], lhsT=oh[:],
                                         rhs=v[:, 0:vw],
                                         start=(b == 0), stop=(b == nb - 1))

                    # ---------------- tile finalize ----------------
                    cp = sb.tile([P, 6], FP32, tag="cp")
                    nc.vector.tensor_copy(cp[:], acc[:, HC:HC + 6])
                    den = sb.tile([P, 2], FP32, tag="den")
                    nc.vector.tensor_scalar_add(den[:], cp[:, 0:2], 1e-16)
                    rden = sb.tile([P, 2], FP32, tag="rden")
                    nc.vector.reciprocal(rden[:], den[:])
                    # ddr = den/(den+eps), swr = swp/(den+eps), both negated
                    nfac = sb.tile([P, 4], FP32, tag="nfac")
                    nc.vector.tensor_mul(nfac[:, 0:2], cp[:, 0:2], rden[:])
                    nc.vector.tensor_mul(nfac[:, 2:4], cp[:, 2:4], rden[:])
                    nc.vector.tensor_scalar_mul(nfac[:], nfac[:], -1.0)
                    hp = sb.tile([P, HC], FP32, tag="hp")
                    c1 = sb.tile([P, HC], FP32, tag="c1")
                    for hh in range(2):
                        cs = slice(hh * CH, (hh + 1) * CH)
                        nc.vector.tensor_scalar_mul(c1[:, cs], acc[:, cs],
                                                    rden[:, hh:hh + 1])
                        nc.vector.scalar_tensor_tensor(
                            out=c1[:, cs], in0=xr_sb[:, cs],
                            scalar=nfac[:, hh:hh + 1], in1=c1[:, cs],
                            op0=OP.mult, op1=OP.add)
                        nc.vector.scalar_tensor_tensor(
                            out=hp[:, cs], in0=werep[:, cs],
                            scalar=nfac[:, 2 + hh:3 + hh], in1=c1[:, cs],
                            op0=OP.mult, op1=OP.add)
                    if layer == 1:
                        ivd = sb.tile([P, 1], FP32, tag="ivd")
                        nc.sync.dma_start(out=ivd[:],
                                          in_=invdT[t * P:(t + 1) * P, :])
                        lat = sb.tile([P, 1], FP32, tag="lat")
                        nc.vector.tensor_mul(lat[:], cp[:, 4:5], ivd[:])
                        slt = sb.tile([P, 1], I32, tag="slt")
                        nc.sync.dma_start(out=slt[:],
                                          in_=slotsT[t * P:(t + 1) * P, :])
                        nc.gpsimd.indirect_dma_start(
                            out=w2i[:, :],
                            out_offset=bass.IndirectOffsetOnAxis(
                                ap=slt[:, :1], axis=0),
                            in_=lat[:], in_offset=None)
                    # stats
                    sq = sb.tile([P, HC], FP32, tag="sq")
                    nc.scalar.activation(sq[:], hp[:], AF.Square)
                    s1_ps = mmp.tile([P, HC], FP32, tag="mm")
                    nc.tensor.matmul(out=s1_ps[0:1, :], lhsT=ones_col[:],
                                     rhs=hp[:], start=True, stop=True)
                    s2_ps = mmp.tile([P, HC], FP32, tag="mm")
                    nc.tensor.matmul(out=s2_ps[0:1, :], lhsT=ones_col[:],
                                     rhs=sq[:], start=True, stop=True)
                    nc.vector.tensor_add(stsb[0:1, 0:HC], stsb[0:1, 0:HC],
                                         s1_ps[0:1, :])
                    nc.vector.tensor_add(stsb[0:1, HC:2 * HC],
                                         stsb[0:1, HC:2 * HC], s2_ps[0:1, :])
                    nc.sync.dma_start(out=hpre_dram[t * P:(t + 1) * P, :],
                                      in_=hp[:])

            # ============ BN finalize: AllReduce stats + scale/shift ============
            def bn_scale_shift(stsb, sti, sto, grow_s, brow_s):
                nc.sync.dma_start(out=sti[:, :], in_=stsb[:])
                nc.gpsimd.collective_compute(
                    "AllReduce", OP.add, ins=[sti[:, :]], outs=[sto[:, :]],
                    replica_groups=RG)
                stg = sb.tile([1, 2 * HC], FP32, tag="stg")
                nc.sync.dma_start(out=stg[:], in_=sto[:, :])
                mu = sb.tile([1, HC], FP32, tag="mu")
                nc.vector.tensor_scalar_mul(mu[:], stg[0:1, 0:HC], 1.0 / N)
                msq = sb.tile([1, HC], FP32, tag="msq")
                nc.vector.tensor_scalar_mul(msq[:], stg[0:1, HC:2 * HC], 1.0 / N)
                musq = sb.tile([1, HC], FP32, tag="musq")
                nc.vector.tensor_mul(musq[:], mu[:], mu[:])
                var = sb.tile([1, HC], FP32, tag="var")
                nc.vector.tensor_sub(var[:], msq[:], musq[:])
                nc.vector.tensor_scalar_add(var[:], var[:], 1e-5)
                sd = sb.tile([1, HC], FP32, tag="sd")
                nc.scalar.activation(sd[:], var[:], AF.Sqrt)
                rsd = sb.tile([1, HC], FP32, tag="rsd")
                nc.vector.reciprocal(rsd[:], sd[:])
                scl = sb.tile([1, HC], FP32, tag="scl")
                nc.vector.tensor_mul(scl[:], grow_s[:], rsd[:])
                sclmu = sb.tile([1, HC], FP32, tag="sclmu")
                nc.vector.tensor_mul(sclmu[:], scl[:], mu[:])
                shf = sb.tile([1, HC], FP32, tag="shf")
                nc.vector.tensor_sub(shf[:], brow_s[:], sclmu[:])
                scl_ps = mmp.tile([P, HC], FP32, tag="mm")
                nc.tensor.matmul(out=scl_ps[:], lhsT=ones_row[:], rhs=scl[:],
                                 start=True, stop=True)
                sclb = cst.tile([P, HC], FP32, tag=f"sclb{id(stsb)}")
                nc.scalar.activation(sclb[:], scl_ps[:], AF.Copy)
                shf_ps = mmp.tile([P, HC], FP32, tag="mm")
                nc.tensor.matmul(out=shf_ps[:], lhsT=ones_row[:], rhs=shf[:],
                                 start=True, stop=True)
                shfb = cst.tile([P, HC], FP32, tag=f"shfb{id(stsb)}")
                nc.scalar.activation(shfb[:], shf_ps[:], AF.Copy)
                return sclb, shfb

            # ================= run the whole network =================
            edge_layer(1)
            scl1b, shf1b = bn_scale_shift(st1sb, st1i, st1o, g1s, be1s)

            # BN1 apply + leaky relu + dropout mask -> h1act (bf16),
            # fused with the layer-2 XR precompute (overlaps the AllGather)
            xr2_all = []
            for t in range(NT):
                ht = sb.tile([P, HC], FP32, tag="ht")
                nc.sync.dma_start(out=ht[:], in_=hpre1[t * P:(t + 1) * P, :])
                t1 = sb.tile([P, HC], FP32, tag="t1")
                nc.vector.tensor_mul(t1[:], ht[:], scl1b[:])
                nc.vector.tensor_add(t1[:], t1[:], shf1b[:])
                ha = sb.tile([P, HC], FP32, tag="ha")
                nc.scalar.activation(ha[:], t1[:], AF.Prelu, alpha=0.01)
                mk = sb.tile([P, HC], FP32, tag="mk")
                nc.sync.dma_start(out=mk[:], in_=mask1T[t * P:(t + 1) * P, :])
                hm = sb.tile([P, HC], BF16, tag="hm")
                nc.vector.tensor_mul(hm[:], ha[:], mk[:])
                xoTs = sb.tile([P, HC], BF16, tag="xoTs2")
                for kk in range(2):
                    tp = trp.tile([P, P], BF16, tag="tr")
                    nc.tensor.transpose(out=tp[:],
                                        in_=hm[:, kk * P:(kk + 1) * P],
                                        identity=identb[:])
                    nc.scalar.activation(xoTs[:, kk * P:(kk + 1) * P], tp[:],
                                         AF.Copy)
                xr_ps = mmp.tile([P, HC], FP32, tag="mm")
                for kk in range(2):
                    nc.tensor.matmul(out=xr_ps[:],
                                     lhsT=xoTs[:, kk * P:(kk + 1) * P],
                                     rhs=Wr2s[:, kk * HC:(kk + 1) * HC],
                                     start=(kk == 0), stop=(kk == 1))
                xr2_t = cst.tile([P, HC], BF16, tag=f"xr2_{t}")
                nc.scalar.activation(xr2_t[:], xr_ps[:], AF.Copy)
                xr2_all.append(xr2_t)
                xl2_ps = mmp.tile([P, HC], FP32, tag="mm")
                for kk in range(2):
                    nc.tensor.matmul(out=xl2_ps[:],
                                     lhsT=xoTs[:, kk * P:(kk + 1) * P],
                                     rhs=Wl2s[:, kk * HC:(kk + 1) * HC],
                                     start=(kk == 0), stop=(kk == 1))
                xl2_sb = sb.tile([P, HC], BF16, tag="xl2_sb")
                nc.scalar.activation(xl2_sb[:], xl2_ps[:], AF.Copy)
                nc.sync.dma_start(out=xl2loc[t * P:(t + 1) * P, :], in_=xl2_sb[:])
                if h1dbg is not None:
                    hmf = sb.tile([P, HC], FP32, tag="hmf")
                    nc.vector.tensor_copy(hmf[:], hm[:])
                    nc.sync.dma_start(out=h1dbg[t * P:(t + 1) * P, :], in_=hmf[:])

            # all-gather the transformed layer-2 source features (bf16)
            nc.gpsimd.collective_compute(
                "AllGather", OP.bypass, ins=[xl2loc[:, :]], outs=[xl2full[:, :]],
                replica_groups=RG)

            edge_layer(2)
            scl2b, shf2b = bn_scale_shift(st2sb, st2i, st2o, g2s, be2s)

            # BN2 apply + leaky relu + dropout + FC
            for t in range(NT):
                ht = sb.tile([P, HC], FP32, tag="ht2")
                nc.sync.dma_start(out=ht[:], in_=hpre2[t * P:(t + 1) * P, :])
                t1 = sb.tile([P, HC], FP32, tag="t12")
                nc.vector.tensor_mul(t1[:], ht[:], scl2b[:])
                nc.vector.tensor_add(t1[:], t1[:], shf2b[:])
                ha = sb.tile([P, HC], FP32, tag="ha2")
                nc.scalar.activation(ha[:], t1[:], AF.Prelu, alpha=0.01)
                mk = sb.tile([P, HC], FP32, tag="mk2")
                nc.sync.dma_start(out=mk[:], in_=mask2T[t * P:(t + 1) * P, :])
                hm = sb.tile([P, HC], BF16, tag="hm2")
                nc.vector.tensor_mul(hm[:], ha[:], mk[:])
                h2T = sb.tile([P, HC], BF16, tag="h2T")
                for kk in range(2):
                    tp = trp.tile([P, P], BF16, tag="tr")
                    nc.tensor.transpose(out=tp[:],
                                        in_=hm[:, kk * P:(kk + 1) * P],
                                        identity=identb[:])
                    nc.vector.tensor_copy(h2T[:, kk * P:(kk + 1) * P], tp[:])
                fc_ps = mmp.tile([P, HC], FP32, tag="mm")
                for kk in range(2):
                    nc.tensor.matmul(out=fc_ps[:, 0:NCF],
                                     lhsT=h2T[:, kk * P:(kk + 1) * P],
                                     rhs=wfcs[:, kk * NCF:(kk + 1) * NCF],
                                     start=(kk == 0), stop=(kk == 1))
                ob = sb.tile([P, NCF], FP32, tag="ob")
                nc.vector.tensor_add(ob[:], fc_ps[:, 0:NCF], bfcs[:])
                nc.sync.dma_start(out=outT[t * P:(t + 1) * P, :], in_=ob[:])

    nc.compile()
    return nc


# ----------------------------------------------------------------------------
# entry point
# ----------------------------------------------------------------------------

def kernel(x_input, edge_weight, params, edge_index):
    in_maps, meta = _host_prep(x_input, edge_weight, params, edge_index)
    key = tuple(sorted(meta.items()))
    if key not in _PROGRAM_CACHE:
        _PROGRAM_CACHE[key] = _build_program(meta)
    nc = _PROGRAM_CACHE[key]
    res = run_bass_kernel_spmd(nc, in_maps, core_ids=list(range(NCORES)))
    if os.environ.get("KERNEL_DBG"):
        kernel.last_res = res
        kernel.last_meta = meta
    if res.exec_time_ns is not None:
        print(f"HW exec time: {res.exec_time_ns} ns")
    out = np.empty((N, NCF), np.float32)
    for r in range(NCORES):
        out[r * NOWN:(r + 1) * NOWN] = res.results[r]["out"][:NOWN]
    return out


# revision 28
# speedup vs baseline: 1.0660x; 1.0660x over previous
"""Trainium2 Bass kernel for nn_AttnGCN (2-layer GATv2 + BN + dropout + FC).

Sharding: nodes are partitioned across 8 NeuronCores (graph parallel).  Each
core owns a contiguous range of 6250 destination nodes (padded to 6272 =
49*128).  Edges are bucketed by destination tile on the host (index-only
preprocessing), each tile's edge list padded to whole 128-edge blocks.  Layer-1
runs per-core on the edge shard; BN statistics are combined with a tiny
AllReduce; the activated layer-1 features are AllGathered (bf16) so every core
can gather arbitrary source rows for layer-2; layer-2 + FC produce the owned
output shard, which the host concatenates.

All numeric work (matmuls, softmax, scatter/gather, BN, masking) happens on
device.  Host does only index bucketing, parameter layout, and output
reassembly.  Dropout masks are the fixed jax PRNG streams of the reference
(input-independent constants), computed once on host CPU.
"""

import os
import sys
import types
import numpy as np
import ml_dtypes

import concourse.bacc as bacc
import concourse.bass as bass
import concourse.mybir as mybir
import concourse.tile as tile
from concourse.bass_utils import run_bass_kernel_spmd
from concourse.masks import make_identity

P = 128
NCORES = 8
N = 50000
E = 400000
NCF = 26          # input/output feature dim
H = 2
CH = 128
HC = 256
NOWN = N // NCORES            # 6250 owned nodes per core
NT = (NOWN + P - 1) // P      # 49 node tiles per core
NPC = NT * P                  # 6272 padded nodes per core
NPAD = NCORES * NPC           # 50176
NREAL_LAST = NOWN - (NT - 1) * P   # 106 real nodes in last tile
DP_SCALE = 1.25               # 1/(1-0.2)
KA = NCF + 1                  # 27: x features + edge weight (for w*We fold)

FP32 = mybir.dt.float32
F32R = mybir.dt.float32r
BF16 = mybir.dt.bfloat16
I32 = mybir.dt.int32
AF = mybir.ActivationFunctionType
OP = mybir.AluOpType
RG = [list(range(NCORES))]
BF = ml_dtypes.bfloat16

_PROGRAM_CACHE = {}
_MASK_CACHE = {}

_MASK_SCRIPT = r"""
import os, sys
for _p in reversed(os.environ.get("NIX_PYTHONPATH", "").split(os.pathsep)):
    if _p and _p not in sys.path:
        sys.path.insert(0, _p)
import numpy as np
import jax
m1 = np.asarray(jax.random.bernoulli(jax.random.key(1), 0.8, (%d, %d)),
                dtype=np.float32)
m2 = np.asarray(jax.random.bernoulli(jax.random.key(2), 0.8, (%d, %d)),
                dtype=np.float32)
np.savez(sys.argv[1], m1=m1, m2=m2)
"""


def _dropout_masks():
    """Reference dropout masks: fixed jax PRNG streams, computed with plain
    CPU jax (subprocess) so the bit stream matches a stock jax environment."""
    if "m" not in _MASK_CACHE:
        import subprocess
        import tempfile
        env = dict(os.environ)
        env["JAX_PLATFORMS"] = "cpu"
        env.pop("XLA_FLAGS", None)
        env.pop("TRN_TERMINAL_POOL_IPS", None)
        with tempfile.TemporaryDirectory() as td:
            fn = os.path.join(td, "masks.npz")
            script = _MASK_SCRIPT % (N, HC, N, HC)
            r = subprocess.run([sys.executable, "-c", script, fn], env=env,
                               capture_output=True, text=True)
            if r.returncode != 0:
                raise RuntimeError("mask subprocess failed: " + r.stderr[-2000:])
            d = np.load(fn)
            _MASK_CACHE["m"] = (d["m1"] * DP_SCALE, d["m2"] * DP_SCALE)
    return _MASK_CACHE["m"]


# ----------------------------------------------------------------------------
# host-side index preprocessing (sharding)
# ----------------------------------------------------------------------------

def _host_prep(x_input, edge_weight, params, edge_index):
    src = np.asarray(edge_index[0], dtype=np.int64).astype(np.int32)
    dst = np.asarray(edge_index[1], dtype=np.int64).astype(np.int32)
    w = np.asarray(edge_weight, dtype=np.float32).reshape(-1)
    x = np.asarray(x_input, dtype=np.float32)

    r_arr = dst // NOWN
    nloc = dst - r_arr * NOWN
    t_arr = nloc >> 7
    dstloc = (nloc & 127).astype(np.int32)

    key = r_arr * NT + t_arr
    order = np.argsort(key, kind="stable")
    cnt = np.bincount(key, minlength=NCORES * NT).reshape(NCORES, NT)
    bounds = np.concatenate([[0], np.cumsum(cnt.reshape(-1))]).astype(np.int64)

    # ----- layer 1 blocks -----
    NBT1 = np.maximum(1, -(-cnt.max(0) // P)).astype(np.int64)
    off1 = np.concatenate([[0], np.cumsum(NBT1[:-1])]).astype(np.int64)
    LB1 = int(NBT1.sum())
    LE1 = LB1 * P
    edges1 = np.zeros((NCORES, LE1, 3), np.int32)
    edges1[:, :, 1] = 255

    for r in range(NCORES):
        for t in range(NT):
            k = r * NT + t
            c = int(cnt[r, t])
            if c:
                sl = order[bounds[k]:bounds[k] + c]
                base = int(off1[t]) * P
                edges1[r, base:base + c, 0] = src[sl]
                edges1[r, base:base + c, 1] = dstloc[sl]
                edges1[r, base:base + c, 2] = w[sl].view(np.int32)

    # ----- layer 2 blocks (real edges + self loops) -----
    nreal = np.full(NT, P, np.int64)
    nreal[NT - 1] = NREAL_LAST
    cnt2 = cnt + nreal[None, :]
    NBT2 = np.maximum(1, -(-cnt2.max(0) // P)).astype(np.int64)
    off2 = np.concatenate([[0], np.cumsum(NBT2[:-1])]).astype(np.int64)
    LB2 = int(NBT2.sum())
    LE2 = LB2 * P
    LE2P = LE2 + P
    g2src = (src // NOWN) * NPC + src % NOWN   # padded-global source ids

    edges2 = np.zeros((NCORES, LE2, 2), np.int32)
    edges2[:, :, 1] = 255
    w2x = np.zeros((NCORES, LE2P), np.float32)
    slots2 = np.zeros((NCORES, NT * P), np.int32)

    for r in range(NCORES):
        for t in range(NT):
            k = r * NT + t
            c = int(cnt[r, t])
            sl = order[bounds[k]:bounds[k] + c]
            base = int(off2[t]) * P
            edges2[r, base:base + c, 0] = g2src[sl]
            edges2[r, base:base + c, 1] = dstloc[sl]
            w2x[r, base:base + c] = w[sl]
            nr = int(nreal[t])
            pos = base + c
            edges2[r, pos:pos + nr, 0] = r * NPC + t * P + np.arange(nr)
            edges2[r, pos:pos + nr, 1] = np.arange(nr)
            slots2[r, t * P:t * P + nr] = pos + np.arange(nr)
            slots2[r, t * P + nr:(t + 1) * P] = LE2 + np.arange(nr, P)

    # ----- degrees / masks / params -----
    deg = np.bincount(dst, minlength=N).astype(np.float32)
    invdeg_full = 1.0 / np.maximum(deg, 1.0)
    invdeg = np.ones((NCORES, NPC), np.float32)
    for r in range(NCORES):
        invdeg[r, :NOWN] = invdeg_full[r * NOWN:(r + 1) * NOWN]

    m1, m2 = _dropout_masks()

    def shard_rows(a, dtype=np.float32):
        out = np.zeros((NCORES, NPC) + a.shape[1:], dtype)
        for r in range(NCORES):
            out[r, :NOWN] = a[r * NOWN:(r + 1) * NOWN]
        return out

    mask1 = shard_rows(m1)
    mask2 = shard_rows(m2)
    x_own = shard_rows(x, BF)

    p = {k: np.asarray(v, dtype=np.float32) for k, v in params.items()}
    wfc_pk = np.zeros((P, 52), BF)
    wfc_pk[:, :26] = p["Wfc"][:128].astype(BF)
    wfc_pk[:, 26:] = p["Wfc"][128:].astype(BF)
    common = {
        "xtb": x.T.astype(BF).copy(),
        "Wl1": p["Wl1"].astype(BF),
        "we1row": p["We1"].reshape(1, HC).astype(BF),
        "Wr1": p["Wr1"].astype(BF),
        "att1r": np.tile(p["att1"].reshape(1, HC), (P, 1)).astype(BF),
        "Wl2": p["Wl2"].astype(BF), "Wr2": p["Wr2"].astype(BF),
        "we2row": p["We2"].reshape(1, HC).astype(BF),
        "we1r": np.tile(p["We1"].reshape(1, HC), (P, 1)),
        "we2r": np.tile(p["We2"].reshape(1, HC), (P, 1)),
        "att2r": np.tile(p["att2"].reshape(1, HC), (P, 1)).astype(BF),
        "wfc": wfc_pk,
        "bfcr": np.tile(p["bfc"].reshape(1, NCF), (P, 1)),
        "g1row": p["g1"].reshape(1, HC), "be1row": p["be1"].reshape(1, HC),
        "g2row": p["g2"].reshape(1, HC), "be2row": p["be2"].reshape(1, HC),
    }
    in_maps = []
    for r in range(NCORES):
        m = dict(common)
        m["edges1"] = edges1[r]
        m["w1c"] = edges1[r][:, 2].view(np.float32).copy()[:, None]
        m["edges2"] = edges2[r]
        m["w2x"] = w2x[r][:, None]
        m["slots2"] = slots2[r][:, None]
        m["invdeg"] = invdeg[r][:, None]
        m["mask1"] = mask1[r]
        m["mask2"] = mask2[r]
        m["x_own"] = x_own[r]
        in_maps.append(m)

    meta = dict(NBT1=tuple(int(v) for v in NBT1), off1=tuple(int(v) for v in off1),
                NBT2=tuple(int(v) for v in NBT2), off2=tuple(int(v) for v in off2),
                LE1=LE1, LE2=LE2, LE2P=LE2P)
    return in_maps, meta


# ----------------------------------------------------------------------------
# device program
# ----------------------------------------------------------------------------

def _build_program(meta):
    NBT1, off1 = meta["NBT1"], meta["off1"]
    NBT2, off2 = meta["NBT2"], meta["off2"]
    LE1, LE2, LE2P = meta["LE1"], meta["LE2"], meta["LE2P"]

    nc = bacc.Bacc("TRN2", target_bir_lowering=False)

    # ---- I/O ----
    xtbT = nc.dram_tensor("xtb", (NCF, N), BF16, kind="ExternalInput")
    e1T = nc.dram_tensor("edges1", (LE1, 3), I32, kind="ExternalInput")
    e2T = nc.dram_tensor("edges2", (LE2, 2), I32, kind="ExternalInput")
    w2xT = nc.dram_tensor("w2x", (LE2P, 1), FP32, kind="ExternalInput")
    slotsT = nc.dram_tensor("slots2", (NT * P, 1), I32, kind="ExternalInput")
    invdT = nc.dram_tensor("invdeg", (NPC, 1), FP32, kind="ExternalInput")
    mask1T = nc.dram_tensor("mask1", (NPC, HC), FP32, kind="ExternalInput")
    mask2T = nc.dram_tensor("mask2", (NPC, HC), FP32, kind="ExternalInput")
    xownT = nc.dram_tensor("x_own", (NPC, NCF), BF16, kind="ExternalInput")
    Wl1T = nc.dram_tensor("Wl1", (NCF, HC), BF16, kind="ExternalInput")
    we1rowT = nc.dram_tensor("we1row", (1, HC), BF16, kind="ExternalInput")
    w1cT = nc.dram_tensor("w1c", (LE1, 1), FP32, kind="ExternalInput")
    Wr1T = nc.dram_tensor("Wr1", (NCF, HC), BF16, kind="ExternalInput")
    att1rT = nc.dram_tensor("att1r", (P, HC), BF16, kind="ExternalInput")
    Wl2T = nc.dram_tensor("Wl2", (HC, HC), BF16, kind="ExternalInput")
    Wr2T = nc.dram_tensor("Wr2", (HC, HC), BF16, kind="ExternalInput")
    we2rowT = nc.dram_tensor("we2row", (1, HC), BF16, kind="ExternalInput")
    we1rT = nc.dram_tensor("we1r", (P, HC), FP32, kind="ExternalInput")
    we2rT = nc.dram_tensor("we2r", (P, HC), FP32, kind="ExternalInput")
    att2rT = nc.dram_tensor("att2r", (P, HC), BF16, kind="ExternalInput")
    wfcT = nc.dram_tensor("wfc", (P, 52), BF16, kind="ExternalInput")
    bfcrT = nc.dram_tensor("bfcr", (P, NCF), FP32, kind="ExternalInput")
    g1rT = nc.dram_tensor("g1row", (1, HC), FP32, kind="ExternalInput")
    be1rT = nc.dram_tensor("be1row", (1, HC), FP32, kind="ExternalInput")
    g2rT = nc.dram_tensor("g2row", (1, HC), FP32, kind="ExternalInput")
    be2rT = nc.dram_tensor("be2row", (1, HC), FP32, kind="ExternalInput")
    outT = nc.dram_tensor("out", (NPC, NCF), FP32, kind="ExternalOutput")

    # ---- internal DRAM ----
    dbg = bool(os.environ.get("KERNEL_DBG"))
    dbgkind = {"kind": "ExternalOutput"} if dbg else {}
    NXL1 = ((N + P - 1) // P) * P
    hpre1 = nc.dram_tensor("hpre1", (NPC, HC), FP32, **dbgkind)
    h1dbg = (nc.dram_tensor("h1dbg", (NPC, HC), FP32, kind="ExternalOutput")
             if dbg else None)
    xl1 = nc.dram_tensor("xl1", (NXL1, HC), BF16)
    xl2loc = nc.dram_tensor("xl2loc", (NPC, HC), BF16)
    xl2full = nc.dram_tensor("xl2full", (NPAD, HC), BF16, addr_space="Shared")
    hpre2 = nc.dram_tensor("hpre2", (NPC, HC), FP32, **dbgkind)
    w2i = nc.dram_tensor("w2i", (LE2P, 1), FP32, **dbgkind)
    st1i = nc.dram_tensor("st1i", (1, 2 * HC), FP32)
    st1o = nc.dram_tensor("st1o", (1, 2 * HC), FP32, addr_space="Shared")
    st2i = nc.dram_tensor("st2i", (1, 2 * HC), FP32)
    st2o = nc.dram_tensor("st2o", (1, 2 * HC), FP32, addr_space="Shared")

    with tile.TileContext(nc) as tc:
        with tc.tile_pool(name="cst", bufs=1) as cst, \
             tc.tile_pool(name="sb", bufs=4) as sb, \
             tc.tile_pool(name="mm", bufs=4, space="PSUM") as mmp, \
             tc.tile_pool(name="tr", bufs=2, space="PSUM") as trp, \
             tc.tile_pool(name="accp", bufs=2, space="PSUM") as accp:

            # ---------------- constants ----------------
            iota_i = cst.tile([P, P], I32, tag="iota_i")
            nc.gpsimd.iota(iota_i[:], pattern=[[1, P]], base=0,
                           channel_multiplier=0)
            iota_f = cst.tile([P, P], FP32, tag="iota_f")
            nc.vector.tensor_copy(iota_f[:], iota_i[:])
            ident32 = cst.tile([P, P], FP32, tag="ident32")
            make_identity(nc, ident32[:])
            identb = cst.tile([P, P], BF16, tag="identb")
            nc.vector.tensor_copy(identb[:], ident32[:])

            def load_const(name, dram, shape, dtype=FP32):
                t = cst.tile(shape, dtype, tag=name)
                nc.sync.dma_start(out=t[:], in_=dram[:, :])
                return t

            Wl1s = load_const("Wl1s", Wl1T, [NCF, HC], BF16)
            we1rs = load_const("we1rs", we1rowT, [1, HC], BF16)
            Wr1s = load_const("Wr1s", Wr1T, [NCF, HC], BF16)
            att1s = load_const("att1s", att1rT, [P, HC], BF16)
            we2rs = load_const("we2rs", we2rowT, [1, HC], BF16)
            we1rep = load_const("we1rep", we1rT, [P, HC])
            we2rep = load_const("we2rep", we2rT, [P, HC])
            att2s = load_const("att2s", att2rT, [P, HC], BF16)
            wfcs = load_const("wfcs", wfcT, [P, 52], BF16)
            bfcs = load_const("bfcs", bfcrT, [P, NCF])
            g1s = load_const("g1s", g1rT, [1, HC])
            be1s = load_const("be1s", be1rT, [1, HC])
            g2s = load_const("g2s", g2rT, [1, HC])
            be2s = load_const("be2s", be2rT, [1, HC])
            Wl2s = cst.tile([P, 2 * HC], BF16, tag="Wl2s")
            Wr2s = cst.tile([P, 2 * HC], BF16, tag="Wr2s")
            for kk in range(2):
                nc.sync.dma_start(out=Wl2s[:, kk * HC:(kk + 1) * HC],
                                  in_=Wl2T[kk * P:(kk + 1) * P, :])
                nc.sync.dma_start(out=Wr2s[:, kk * HC:(kk + 1) * HC],
                                  in_=Wr2T[kk * P:(kk + 1) * P, :])
            ones_col = cst.tile([P, 1], FP32, tag="ones_col")
            nc.vector.memset(ones_col[:], 1.0)
            ones_row = cst.tile([1, P], FP32, tag="ones_row")
            nc.vector.memset(ones_row[:], 1.0)

            st1sb = cst.tile([1, 2 * HC], FP32, tag="st1sb")
            nc.vector.memset(st1sb[:], 0.0)
            st2sb = cst.tile([1, 2 * HC], FP32, tag="st2sb")
            nc.vector.memset(st2sb[:], 0.0)

            # copy host edge weights for layer 2 (self-loop slots get filled
            # by the device during layer-1 finalize)
            nrows = LE2P // P
            for c0 in range(0, nrows, P):
                cn = min(P, nrows - c0)
                w2cp = sb.tile([P, P], FP32, tag="w2cp")
                nc.sync.dma_start(
                    out=w2cp[:cn, :],
                    in_=w2xT[:, 0].rearrange("(a b) -> a b", b=P)[c0:c0 + cn, :])
                nc.sync.dma_start(
                    out=w2i[:, 0].rearrange("(a b) -> a b", b=P)[c0:c0 + cn, :],
                    in_=w2cp[:cn, :])

            # ---------- XL1 = x @ Wl1 for every node (replicated) ----------
            GRP = 4
            nchunks = (N + P - 1) // P          # 391
            for j0 in range(0, nchunks, GRP):
                jn = min(GRP, nchunks - j0)
                c0 = j0 * P
                cn = min(GRP * P, N - c0)
                xtb_sb = sb.tile([NCF, GRP * P], BF16, tag="xtb_sb")
                nc.sync.dma_start(out=xtb_sb[:, 0:cn],
                                  in_=xtbT[:, c0:c0 + cn])
                xl_sb = sb.tile([P, GRP * HC], BF16, tag="xl_sb")
                for j in range(jn):
                    n0 = (j0 + j) * P
                    nn = min(P, N - n0)
                    xl_ps = mmp.tile([P, HC], FP32, tag="mm")
                    nc.tensor.matmul(out=xl_ps[:nn, :],
                                     lhsT=xtb_sb[:, j * P:j * P + nn],
                                     rhs=Wl1s[:], start=True, stop=True)
                    nc.vector.tensor_copy(xl_sb[:nn, j * HC:(j + 1) * HC],
                                          xl_ps[:nn, :])
                full = jn * P if (j0 + jn) * P <= N else None
                if full:
                    nc.sync.dma_start(
                        out=xl1[c0:c0 + jn * P, :].rearrange(
                            "(j p) c -> p j c", p=P),
                        in_=xl_sb[:].rearrange(
                            "p (j c) -> p j c", c=HC)[:, 0:jn, :])
                else:
                    for j in range(jn):
                        n0 = (j0 + j) * P
                        nn = min(P, N - n0)
                        nc.sync.dma_start(out=xl1[n0:n0 + nn, :],
                                          in_=xl_sb[:nn, j * HC:(j + 1) * HC])

            # ================= generic GATv2 edge layer =================
            # Per block:  m = G + XR[dst] + w*We  accumulated in ONE psum
            # group on PE.  The destination-scatter aggregates p*m; the
            # XR/We parts are removed per-tile:
            #   num = S - xr (x) den - We (x) sum(p*w),   h = num / den.
            def edge_layer(layer):
                if layer == 1:
                    NBT, off, eT = NBT1, off1, e1T
                    atts = att1s
                    hpre_dram = hpre1
                    stsb = st1sb
                    ew = 3   # ints per edge record
                    wers, werep = we1rs, we1rep
                    vw = 262
                else:
                    NBT, off, eT = NBT2, off2, e2T
                    atts = att2s
                    hpre_dram = hpre2
                    stsb = st2sb
                    ew = 2
                    wers, werep = we2rs, we2rep
                    vw = 260

                for t in range(NT):
                    nb = NBT[t]
                    # ---- XR tile for the 128 owned nodes ----
                    if layer == 1:
                        xo = sb.tile([P, NCF], BF16, tag="xo")
                        nc.sync.dma_start(out=xo[:],
                                          in_=xownT[t * P:(t + 1) * P, :])
                        xoT_ps = trp.tile([P, P], BF16, tag="tr")
                        nc.tensor.transpose(out=xoT_ps[:NCF, :], in_=xo[:],
                                            identity=identb[:])
                        xoTs = sb.tile([NCF, P], BF16, tag="xoTs")
                        nc.vector.tensor_copy(xoTs[:], xoT_ps[:NCF, :])
                        xr_ps = mmp.tile([P, HC], FP32, tag="mm")
                        nc.tensor.matmul(out=xr_ps[:], lhsT=xoTs[:],
                                         rhs=Wr1s[:], start=True, stop=True)
                        xr_sb = sb.tile([P, HC], BF16, tag="xr_sb")
                        nc.scalar.activation(xr_sb[:], xr_ps[:], AF.Copy)
                    else:
                        xr_sb = xr2_all[t]

                    # ---- whole tile's edge records in one DMA ----
                    o0 = off[t] * P
                    ebt = sb.tile([P, nb * ew], I32, tag="ebt")
                    nc.sync.dma_start(
                        out=ebt[:].rearrange("p (b c) -> p b c", c=ew),
                        in_=eT[o0:o0 + nb * P, :].rearrange(
                            "(b p) c -> p b c", p=P))
                    wsrc = w1cT if layer == 1 else w2i
                    wrf = sb.tile([1, nb * P], FP32, tag="wrf")
                    nc.sync.dma_start(
                        out=wrf[:],
                        in_=wsrc[o0:o0 + nb * P, :].rearrange("a one -> one a"))
                    wrowt = sb.tile([1, nb * P], BF16, tag="wrowt")
                    nc.vector.tensor_copy(wrowt[:], wrf[:])
                    # per-partition w and float dst for the whole tile
                    wpp = sb.tile([P, nb], FP32, tag="wpp")
                    nc.sync.dma_start(
                        out=wpp[:],
                        in_=wsrc[:, 0].rearrange(
                            "(a p) -> p a", p=P)[:, off[t]:off[t] + nb])
                    d_ft = sb.tile([P, nb], FP32, tag="d_ft")
                    nc.vector.tensor_copy(
                        d_ft[:], ebt[:].rearrange("p (b c) -> p b c", c=ew)[:, :, 1])

                    acc = accp.tile([P, 262], FP32, tag="acc")
                    for b in range(nb):
                        sidx = ebt[:, b * ew:b * ew + 1]

                        # ---- one-hot by local destination ----
                        oh = sb.tile([P, P], BF16, tag="oh")
                        nc.vector.tensor_scalar(
                            out=oh[:], in0=iota_f[:], scalar1=d_ft[:, b:b + 1],
                            scalar2=None, op0=OP.is_equal)
                        ohT_ps = trp.tile([P, P], BF16, tag="tr")
                        nc.tensor.transpose(out=ohT_ps[:], in_=oh[:],
                                            identity=identb[:])
                        ohTs = sb.tile([P, P], BF16, tag="ohTs")
                        nc.scalar.activation(ohTs[:], ohT_ps[:], AF.Copy)

                        # ---- gather pre-transformed source rows ----
                        gsrc = xl1 if layer == 1 else xl2full
                        grow = sb.tile([P, HC], BF16, tag="grow")
                        nc.gpsimd.indirect_dma_start(
                            out=grow[:], out_offset=None, in_=gsrc[:, :],
                            in_offset=bass.IndirectOffsetOnAxis(
                                ap=sidx, axis=0))
                        # ---- m = G + XR[dst] + w*We in one psum group ----
                        m_ps = mmp.tile([P, HC], FP32, tag="mm")
                        nc.tensor.matmul(out=m_ps[:], lhsT=identb[:],
                                         rhs=grow[:], start=True, stop=False)
                        nc.tensor.matmul(out=m_ps[:], lhsT=ohTs[:],
                                         rhs=xr_sb[:], start=False, stop=False)
                        nc.tensor.matmul(out=m_ps[:],
                                         lhsT=wrowt[:, b * P:(b + 1) * P],
                                         rhs=wers[:], start=False, stop=True)

                        # ---- attention logits from Prelu(m) ----
                        lrm = sb.tile([P, HC], BF16, tag="lrm")
                        nc.scalar.activation(lrm[:], m_ps[:], AF.Prelu,
                                             alpha=0.2)
                        junk = sb.tile([P, P], BF16, tag="junk")
                        alpha = sb.tile([P, 2], FP32, tag="alpha")
                        for hh in range(2):
                            nc.vector.scalar_tensor_tensor(
                                out=junk[:],
                                in0=lrm[:, hh * CH:(hh + 1) * CH], scalar=1.0,
                                in1=atts[:, hh * CH:(hh + 1) * CH],
                                op0=OP.mult, op1=OP.mult,
                                accum_out=alpha[:, hh:hh + 1])
                        pexp = sb.tile([P, 2], FP32, tag="pexp")
                        nc.scalar.activation(pexp[:], alpha[:], AF.Exp)

                        # ---- V = [p*m | p | p*w | (w,w)]; scatter by dst ----
                        v = sb.tile([P, 262], BF16, tag="v")
                        nc.scalar.activation(v[:, 0:CH], m_ps[:, 0:CH],
                                             AF.Copy, scale=pexp[:, 0:1])
                        nc.scalar.activation(v[:, CH:HC], m_ps[:, CH:HC],
                                             AF.Copy, scale=pexp[:, 1:2])
                        nc.vector.tensor_copy(v[:, HC:HC + 2], pexp[:])
                        nc.vector.tensor_scalar_mul(v[:, HC + 2:HC + 4],
                                                    pexp[:], wpp[:, b:b + 1])
                        if layer == 1:
                            nc.vector.tensor_copy(
                                v[:, HC + 4:HC + 6],
                                wpp[:, b:b + 1].to_broadcast([P, 2]))
                        nc.tensor.matmul(out=acc[:, 0:vw], lhsT=oh[:],
                                         rhs=v[:, 0:vw],
                                         start=(b == 0), stop=(b == nb - 1))

                    # ---------------- tile finalize ----------------
                    cp = sb.tile([P, 6], FP32, tag="cp")
                    nc.vector.tensor_copy(cp[:], acc[:, HC:HC + 6])
                    den = sb.tile([P, 2], FP32, tag="den")
                    nc.vector.tensor_scalar_add(den[:], cp[:, 0:2], 1e-16)
                    rden = sb.tile([P, 2], FP32, tag="rden")
                    nc.vector.reciprocal(rden[:], den[:])
                    # ddr = den/(den+eps), swr = swp/(den+eps), both negated
                    nfac = sb.tile([P, 4], FP32, tag="nfac")
                    nc.vector.tensor_mul(nfac[:, 0:2], cp[:, 0:2], rden[:])
                    nc.vector.tensor_mul(nfac[:, 2:4], cp[:, 2:4], rden[:])
                    nc.vector.tensor_scalar_mul(nfac[:], nfac[:], -1.0)
                    hp = sb.tile([P, HC], FP32, tag="hp")
                    c1 = sb.tile([P, HC], FP32, tag="c1")
                    for hh in range(2):
                        cs = slice(hh * CH, (hh + 1) * CH)
                        nc.vector.tensor_scalar_mul(c1[:, cs], acc[:, cs],
                                                    rden[:, hh:hh + 1])
                        nc.vector.scalar_tensor_tensor(
                            out=c1[:, cs], in0=xr_sb[:, cs],
                            scalar=nfac[:, hh:hh + 1], in1=c1[:, cs],
                            op0=OP.mult, op1=OP.add)
                        nc.vector.scalar_tensor_tensor(
                            out=hp[:, cs], in0=werep[:, cs],
                            scalar=nfac[:, 2 + hh:3 + hh], in1=c1[:, cs],
                            op0=OP.mult, op1=OP.add)
                    if layer == 1:
                        ivd = sb.tile([P, 1], FP32, tag="ivd")
                        nc.sync.dma_start(out=ivd[:],
                                          in_=invdT[t * P:(t + 1) * P, :])
                        lat = sb.tile([P, 1], FP32, tag="lat")
                        nc.vector.tensor_mul(lat[:], cp[:, 4:5], ivd[:])
                        slt = sb.tile([P, 1], I32, tag="slt")
                        nc.sync.dma_start(out=slt[:],
                                          in_=slotsT[t * P:(t + 1) * P, :])
                        nc.gpsimd.indirect_dma_start(
                            out=w2i[:, :],
                            out_offset=bass.IndirectOffsetOnAxis(
                                ap=slt[:, :1], axis=0),
                            in_=lat[:], in_offset=None)
                    # stats
                    sq = sb.tile([P, HC], FP32, tag="sq")
                    nc.scalar.activation(sq[:], hp[:], AF.Square)
                    s1_ps = mmp.tile([P, HC], FP32, tag="mm")
                    nc.tensor.matmul(out=s1_ps[0:1, :], lhsT=ones_col[:],
                                     rhs=hp[:], start=True, stop=True)
                    s2_ps = mmp.tile([P, HC], FP32, tag="mm")
                    nc.tensor.matmul(out=s2_ps[0:1, :], lhsT=ones_col[:],
                                     rhs=sq[:], start=True, stop=True)
                    nc.vector.tensor_add(stsb[0:1, 0:HC], stsb[0:1, 0:HC],
                                         s1_ps[0:1, :])
                    nc.vector.tensor_add(stsb[0:1, HC:2 * HC],
                                         stsb[0:1, HC:2 * HC], s2_ps[0:1, :])
                    nc.sync.dma_start(out=hpre_dram[t * P:(t + 1) * P, :],
                                      in_=hp[:])

            # ============ BN finalize: AllReduce stats + scale/shift ============
            def bn_scale_shift(stsb, sti, sto, grow_s, brow_s):
                nc.sync.dma_start(out=sti[:, :], in_=stsb[:])
                nc.gpsimd.collective_compute(
                    "AllReduce", OP.add, ins=[sti[:, :]], outs=[sto[:, :]],
                    replica_groups=RG)
                stg = sb.tile([1, 2 * HC], FP32, tag="stg")
                nc.sync.dma_start(out=stg[:], in_=sto[:, :])
                mu = sb.tile([1, HC], FP32, tag="mu")
                nc.vector.tensor_scalar_mul(mu[:], stg[0:1, 0:HC], 1.0 / N)
                msq = sb.tile([1, HC], FP32, tag="msq")
                nc.vector.tensor_scalar_mul(msq[:], stg[0:1, HC:2 * HC], 1.0 / N)
                musq = sb.tile([1, HC], FP32, tag="musq")
                nc.vector.tensor_mul(musq[:], mu[:], mu[:])
                var = sb.tile([1, HC], FP32, tag="var")
                nc.vector.tensor_sub(var[:], msq[:], musq[:])
                nc.vector.tensor_scalar_add(var[:], var[:], 1e-5)
                sd = sb.tile([1, HC], FP32, tag="sd")
                nc.scalar.activation(sd[:], var[:], AF.Sqrt)
                rsd = sb.tile([1, HC], FP32, tag="rsd")
                nc.vector.reciprocal(rsd[:], sd[:])
                scl = sb.tile([1, HC], FP32, tag="scl")
                nc.vector.tensor_mul(scl[:], grow_s[:], rsd[:])
                sclmu = sb.tile([1, HC], FP32, tag="sclmu")
                nc.vector.tensor_mul(sclmu[:], scl[:], mu[:])
                shf = sb.tile([1, HC], FP32, tag="shf")
                nc.vector.tensor_sub(shf[:], brow_s[:], sclmu[:])
                scl_ps = mmp.tile([P, HC], FP32, tag="mm")
                nc.tensor.matmul(out=scl_ps[:], lhsT=ones_row[:], rhs=scl[:],
                                 start=True, stop=True)
                sclb = cst.tile([P, HC], FP32, tag=f"sclb{id(stsb)}")
                nc.scalar.activation(sclb[:], scl_ps[:], AF.Copy)
                shf_ps = mmp.tile([P, HC], FP32, tag="mm")
                nc.tensor.matmul(out=shf_ps[:], lhsT=ones_row[:], rhs=shf[:],
                                 start=True, stop=True)
                shfb = cst.tile([P, HC], FP32, tag=f"shfb{id(stsb)}")
                nc.scalar.activation(shfb[:], shf_ps[:], AF.Copy)
                return sclb, shfb

            # ================= run the whole network =================
            edge_layer(1)
            scl1b, shf1b = bn_scale_shift(st1sb, st1i, st1o, g1s, be1s)

            # BN1 apply + leaky relu + dropout mask -> h1act (bf16),
            # fused with the layer-2 XR precompute (overlaps the AllGather)
            xr2_all = []
            for t in range(NT):
                ht = sb.tile([P, HC], FP32, tag="ht")
                nc.sync.dma_start(out=ht[:], in_=hpre1[t * P:(t + 1) * P, :])
                t1 = sb.tile([P, HC], FP32, tag="t1")
                nc.vector.tensor_mul(t1[:], ht[:], scl1b[:])
                nc.vector.tensor_add(t1[:], t1[:], shf1b[:])
                ha = sb.tile([P, HC], FP32, tag="ha")
                nc.scalar.activation(ha[:], t1[:], AF.Prelu, alpha=0.01)
                mk = sb.tile([P, HC], FP32, tag="mk")
                nc.sync.dma_start(out=mk[:], in_=mask1T[t * P:(t + 1) * P, :])
                hm = sb.tile([P, HC], BF16, tag="hm")
                nc.vector.tensor_mul(hm[:], ha[:], mk[:])
                xoTs = sb.tile([P, HC], BF16, tag="xoTs2")
                for kk in range(2):
                    tp = trp.tile([P, P], BF16, tag="tr")
                    nc.tensor.transpose(out=tp[:],
                                        in_=hm[:, kk * P:(kk + 1) * P],
                                        identity=identb[:])
                    nc.scalar.activation(xoTs[:, kk * P:(kk + 1) * P], tp[:],
                                         AF.Copy)
                xr_ps = mmp.tile([P, HC], FP32, tag="mm")
                for kk in range(2):
                    nc.tensor.matmul(out=xr_ps[:],
                                     lhsT=xoTs[:, kk * P:(kk + 1) * P],
                                     rhs=Wr2s[:, kk * HC:(kk + 1) * HC],
                                     start=(kk == 0), stop=(kk == 1))
                xr2_t = cst.tile([P, HC], BF16, tag=f"xr2_{t}")
                nc.scalar.activation(xr2_t[:], xr_ps[:], AF.Copy)
                xr2_all.append(xr2_t)
                xl2_ps = mmp.tile([P, HC], FP32, tag="mm")
                for kk in range(2):
                    nc.tensor.matmul(out=xl2_ps[:],
                                     lhsT=xoTs[:, kk * P:(kk + 1) * P],
                                     rhs=Wl2s[:, kk * HC:(kk + 1) * HC],
                                     start=(kk == 0), stop=(kk == 1))
                xl2_sb = sb.tile([P, HC], BF16, tag="xl2_sb")
                nc.scalar.activation(xl2_sb[:], xl2_ps[:], AF.Copy)
                nc.sync.dma_start(out=xl2loc[t * P:(t + 1) * P, :], in_=xl2_sb[:])
                if h1dbg is not None:
                    hmf = sb.tile([P, HC], FP32, tag="hmf")
                    nc.vector.tensor_copy(hmf[:], hm[:])
                    nc.sync.dma_start(out=h1dbg[t * P:(t + 1) * P, :], in_=hmf[:])

            # all-gather the transformed layer-2 source features (bf16)
            nc.gpsimd.collective_compute(
                "AllGather", OP.bypass, ins=[xl2loc[:, :]], outs=[xl2full[:, :]],
                replica_groups=RG)

            edge_layer(2)
            scl2b, shf2b = bn_scale_shift(st2sb, st2i, st2o, g2s, be2s)

            # BN2 apply + leaky relu + dropout + FC
            for t in range(NT):
                ht = sb.tile([P, HC], FP32, tag="ht2")
                nc.sync.dma_start(out=ht[:], in_=hpre2[t * P:(t + 1) * P, :])
                t1 = sb.tile([P, HC], FP32, tag="t12")
                nc.vector.tensor_mul(t1[:], ht[:], scl2b[:])
                nc.vector.tensor_add(t1[:], t1[:], shf2b[:])
                ha = sb.tile([P, HC], FP32, tag="ha2")
                nc.scalar.activation(ha[:], t1[:], AF.Prelu, alpha=0.01)
                mk = sb.tile([P, HC], FP32, tag="mk2")
                nc.sync.dma_start(out=mk[:], in_=mask2T[t * P:(t + 1) * P, :])
                hm = sb.tile([P, HC], BF16, tag="hm2")
                nc.vector.tensor_mul(hm[:], ha[:], mk[:])
                h2T = sb.tile([P, HC], BF16, tag="h2T")
                for kk in range(2):
                    tp = trp.tile([P, P], BF16, tag="tr")
                    nc.tensor.transpose(out=tp[:],
                                        in_=hm[:, kk * P:(kk + 1) * P],
                                        identity=identb[:])
                    nc.vector.tensor_copy(h2T[:, kk * P:(kk + 1) * P], tp[:])
                fc_ps = mmp.tile([P, HC], FP32, tag="mm")
                for kk in range(2):
                    nc.tensor.matmul(out=fc_ps[:, 0:NCF],
                                     lhsT=h2T[:, kk * P:(kk + 1) * P],
                                     rhs=wfcs[:, kk * NCF:(kk + 1) * NCF],
                                     start=(kk == 0), stop=(kk == 1))
                ob = sb.tile([P, NCF], FP32, tag="ob")
                nc.vector.tensor_add(ob[:], fc_ps[:, 0:NCF], bfcs[:])
                nc.sync.dma_start(out=outT[t * P:(t + 1) * P, :], in_=ob[:])

    nc.compile()
    return nc


# ----------------------------------------------------------------------------
# entry point
# ----------------------------------------------------------------------------

def kernel(x_input, edge_weight, params, edge_index):
    in_maps, meta = _host_prep(x_input, edge_weight, params, edge_index)
    key = tuple(sorted(meta.items()))
    if key not in _PROGRAM_CACHE:
        _PROGRAM_CACHE[key] = _build_program(meta)
    nc = _PROGRAM_CACHE[key]
    res = run_bass_kernel_spmd(nc, in_maps, core_ids=list(range(NCORES)))
    if os.environ.get("KERNEL_DBG"):
        kernel.last_res = res
        kernel.last_meta = meta
    if res.exec_time_ns is not None:
        print(f"HW exec time: {res.exec_time_ns} ns")
    out = np.empty((N, NCF), np.float32)
    for r in range(NCORES):
        out[r * NOWN:(r + 1) * NOWN] = res.results[r]["out"][:NOWN]
    return out


# revision 31
# speedup vs baseline: 1.1270x; 1.0572x over previous
"""Trainium2 Bass kernel for nn_AttnGCN (2-layer GATv2 + BN + dropout + FC).

Sharding: nodes are partitioned across 8 NeuronCores (graph parallel).  Each
core owns a contiguous range of 6250 destination nodes (padded to 6272 =
49*128).  Edges are bucketed by destination tile on the host (index-only
preprocessing), each tile's edge list padded to whole 128-edge blocks.  Layer-1
runs per-core on the edge shard; BN statistics are combined with a tiny
AllReduce; the activated layer-1 features are AllGathered (bf16) so every core
can gather arbitrary source rows for layer-2; layer-2 + FC produce the owned
output shard, which the host concatenates.

All numeric work (matmuls, softmax, scatter/gather, BN, masking) happens on
device.  Host does only index bucketing, parameter layout, and output
reassembly.  Dropout masks are the fixed jax PRNG streams of the reference
(input-independent constants), computed once on host CPU.
"""

import os
import sys
import types
import numpy as np
import ml_dtypes

import concourse.bacc as bacc
import concourse.bass as bass
import concourse.mybir as mybir
import concourse.tile as tile
from concourse.bass_utils import run_bass_kernel_spmd
from concourse.masks import make_identity

P = 128
NCORES = 8
N = 50000
E = 400000
NCF = 26          # input/output feature dim
H = 2
CH = 128
HC = 256
NOWN = N // NCORES            # 6250 owned nodes per core
NT = (NOWN + P - 1) // P      # 49 node tiles per core
NPC = NT * P                  # 6272 padded nodes per core
NPAD = NCORES * NPC           # 50176
NREAL_LAST = NOWN - (NT - 1) * P   # 106 real nodes in last tile
DP_SCALE = 1.25               # 1/(1-0.2)
KA = NCF + 1                  # 27: x features + edge weight (for w*We fold)

FP32 = mybir.dt.float32
F32R = mybir.dt.float32r
BF16 = mybir.dt.bfloat16
I32 = mybir.dt.int32
AF = mybir.ActivationFunctionType
OP = mybir.AluOpType
RG = [list(range(NCORES))]
BF = ml_dtypes.bfloat16

_PROGRAM_CACHE = {}
_MASK_CACHE = {}

_MASK_SCRIPT = r"""
import os, sys
for _p in reversed(os.environ.get("NIX_PYTHONPATH", "").split(os.pathsep)):
    if _p and _p not in sys.path:
        sys.path.insert(0, _p)
import numpy as np
import jax
m1 = np.asarray(jax.random.bernoulli(jax.random.key(1), 0.8, (%d, %d)),
                dtype=np.float32)
m2 = np.asarray(jax.random.bernoulli(jax.random.key(2), 0.8, (%d, %d)),
                dtype=np.float32)
np.savez(sys.argv[1], m1=m1, m2=m2)
"""


def _dropout_masks():
    """Reference dropout masks: fixed jax PRNG streams, computed with plain
    CPU jax (subprocess) so the bit stream matches a stock jax environment."""
    if "m" not in _MASK_CACHE:
        import subprocess
        import tempfile
        env = dict(os.environ)
        env["JAX_PLATFORMS"] = "cpu"
        env.pop("XLA_FLAGS", None)
        env.pop("TRN_TERMINAL_POOL_IPS", None)
        with tempfile.TemporaryDirectory() as td:
            fn = os.path.join(td, "masks.npz")
            script = _MASK_SCRIPT % (N, HC, N, HC)
            r = subprocess.run([sys.executable, "-c", script, fn], env=env,
                               capture_output=True, text=True)
            if r.returncode != 0:
                raise RuntimeError("mask subprocess failed: " + r.stderr[-2000:])
            d = np.load(fn)
            _MASK_CACHE["m"] = (d["m1"] * DP_SCALE, d["m2"] * DP_SCALE)
    return _MASK_CACHE["m"]


# ----------------------------------------------------------------------------
# host-side index preprocessing (sharding)
# ----------------------------------------------------------------------------

def _host_prep(x_input, edge_weight, params, edge_index):
    src = np.asarray(edge_index[0], dtype=np.int64).astype(np.int32)
    dst = np.asarray(edge_index[1], dtype=np.int64).astype(np.int32)
    w = np.asarray(edge_weight, dtype=np.float32).reshape(-1)
    x = np.asarray(x_input, dtype=np.float32)

    r_arr = dst // NOWN
    nloc = dst - r_arr * NOWN
    t_arr = nloc >> 7
    dstloc = (nloc & 127).astype(np.int32)

    key = r_arr * NT + t_arr
    order = np.argsort(key, kind="stable")
    cnt = np.bincount(key, minlength=NCORES * NT).reshape(NCORES, NT)
    bounds = np.concatenate([[0], np.cumsum(cnt.reshape(-1))]).astype(np.int64)

    # ----- layer 1 blocks -----
    NBT1 = np.maximum(1, -(-cnt.max(0) // P)).astype(np.int64)
    off1 = np.concatenate([[0], np.cumsum(NBT1[:-1])]).astype(np.int64)
    LB1 = int(NBT1.sum())
    LE1 = LB1 * P
    edges1 = np.zeros((NCORES, LE1, 3), np.int32)
    edges1[:, :, 1] = 255

    for r in range(NCORES):
        for t in range(NT):
            k = r * NT + t
            c = int(cnt[r, t])
            if c:
                sl = order[bounds[k]:bounds[k] + c]
                base = int(off1[t]) * P
                edges1[r, base:base + c, 0] = src[sl]
                edges1[r, base:base + c, 1] = dstloc[sl]
                edges1[r, base:base + c, 2] = w[sl].view(np.int32)

    # ----- layer 2 blocks (real edges + self loops) -----
    nreal = np.full(NT, P, np.int64)
    nreal[NT - 1] = NREAL_LAST
    cnt2 = cnt + nreal[None, :]
    NBT2 = np.maximum(1, -(-cnt2.max(0) // P)).astype(np.int64)
    off2 = np.concatenate([[0], np.cumsum(NBT2[:-1])]).astype(np.int64)
    LB2 = int(NBT2.sum())
    LE2 = LB2 * P
    LE2P = LE2 + P
    g2src = (src // NOWN) * NPC + src % NOWN   # padded-global source ids

    edges2 = np.zeros((NCORES, LE2, 2), np.int32)
    edges2[:, :, 1] = 255
    w2x = np.zeros((NCORES, LE2P), np.float32)
    slots2 = np.zeros((NCORES, NT * P), np.int32)

    for r in range(NCORES):
        for t in range(NT):
            k = r * NT + t
            c = int(cnt[r, t])
            sl = order[bounds[k]:bounds[k] + c]
            base = int(off2[t]) * P
            edges2[r, base:base + c, 0] = g2src[sl]
            edges2[r, base:base + c, 1] = dstloc[sl]
            w2x[r, base:base + c] = w[sl]
            nr = int(nreal[t])
            pos = base + c
            edges2[r, pos:pos + nr, 0] = r * NPC + t * P + np.arange(nr)
            edges2[r, pos:pos + nr, 1] = np.arange(nr)
            slots2[r, t * P:t * P + nr] = pos + np.arange(nr)
            slots2[r, t * P + nr:(t + 1) * P] = LE2 + np.arange(nr, P)

    # ----- degrees / masks / params -----
    deg = np.bincount(dst, minlength=N).astype(np.float32)
    invdeg_full = 1.0 / np.maximum(deg, 1.0)
    invdeg = np.ones((NCORES, NPC), np.float32)
    for r in range(NCORES):
        invdeg[r, :NOWN] = invdeg_full[r * NOWN:(r + 1) * NOWN]

    m1, m2 = _dropout_masks()

    def shard_rows(a, dtype=np.float32):
        out = np.zeros((NCORES, NPC) + a.shape[1:], dtype)
        for r in range(NCORES):
            out[r, :NOWN] = a[r * NOWN:(r + 1) * NOWN]
        return out

    mask1 = shard_rows(m1)
    mask2 = shard_rows(m2)
    x_own = shard_rows(x, BF)

    p = {k: np.asarray(v, dtype=np.float32) for k, v in params.items()}
    wfc_pk = np.zeros((P, 52), BF)
    wfc_pk[:, :26] = p["Wfc"][:128].astype(BF)
    wfc_pk[:, 26:] = p["Wfc"][128:].astype(BF)
    common = {
        "xtb": x.T.astype(BF).copy(),
        "Wl1": p["Wl1"].astype(BF),
        "we1row": p["We1"].reshape(1, HC).astype(BF),
        "Wr1": p["Wr1"].astype(BF),
        "att1r": np.tile(p["att1"].reshape(1, HC), (P, 1)).astype(BF),
        "Wl2": p["Wl2"].astype(BF), "Wr2": p["Wr2"].astype(BF),
        "we2row": p["We2"].reshape(1, HC).astype(BF),
        "we1r": np.tile(p["We1"].reshape(1, HC), (P, 1)),
        "we2r": np.tile(p["We2"].reshape(1, HC), (P, 1)),
        "att2r": np.tile(p["att2"].reshape(1, HC), (P, 1)).astype(BF),
        "wfc": wfc_pk,
        "bfcr": np.tile(p["bfc"].reshape(1, NCF), (P, 1)),
        "g1row": p["g1"].reshape(1, HC), "be1row": p["be1"].reshape(1, HC),
        "g2row": p["g2"].reshape(1, HC), "be2row": p["be2"].reshape(1, HC),
    }
    in_maps = []
    for r in range(NCORES):
        m = dict(common)
        m["edges1"] = edges1[r]
        m["w1c"] = edges1[r][:, 2].view(np.float32).copy()[:, None]
        m["edges2"] = edges2[r]
        m["w2x"] = w2x[r][:, None]
        m["slots2"] = slots2[r][:, None]
        m["invdeg"] = invdeg[r][:, None]
        m["mask1"] = mask1[r]
        m["mask2"] = mask2[r]
        m["x_own"] = x_own[r]
        in_maps.append(m)

    meta = dict(NBT1=tuple(int(v) for v in NBT1), off1=tuple(int(v) for v in off1),
                NBT2=tuple(int(v) for v in NBT2), off2=tuple(int(v) for v in off2),
                LE1=LE1, LE2=LE2, LE2P=LE2P)
    return in_maps, meta


# ----------------------------------------------------------------------------
# device program
# ----------------------------------------------------------------------------

def _build_program(meta):
    NBT1, off1 = meta["NBT1"], meta["off1"]
    NBT2, off2 = meta["NBT2"], meta["off2"]
    LE1, LE2, LE2P = meta["LE1"], meta["LE2"], meta["LE2P"]

    nc = bacc.Bacc("TRN2", target_bir_lowering=False)

    # ---- I/O ----
    xtbT = nc.dram_tensor("xtb", (NCF, N), BF16, kind="ExternalInput")
    e1T = nc.dram_tensor("edges1", (LE1, 3), I32, kind="ExternalInput")
    e2T = nc.dram_tensor("edges2", (LE2, 2), I32, kind="ExternalInput")
    w2xT = nc.dram_tensor("w2x", (LE2P, 1), FP32, kind="ExternalInput")
    slotsT = nc.dram_tensor("slots2", (NT * P, 1), I32, kind="ExternalInput")
    invdT = nc.dram_tensor("invdeg", (NPC, 1), FP32, kind="ExternalInput")
    mask1T = nc.dram_tensor("mask1", (NPC, HC), FP32, kind="ExternalInput")
    mask2T = nc.dram_tensor("mask2", (NPC, HC), FP32, kind="ExternalInput")
    xownT = nc.dram_tensor("x_own", (NPC, NCF), BF16, kind="ExternalInput")
    Wl1T = nc.dram_tensor("Wl1", (NCF, HC), BF16, kind="ExternalInput")
    we1rowT = nc.dram_tensor("we1row", (1, HC), BF16, kind="ExternalInput")
    w1cT = nc.dram_tensor("w1c", (LE1, 1), FP32, kind="ExternalInput")
    Wr1T = nc.dram_tensor("Wr1", (NCF, HC), BF16, kind="ExternalInput")
    att1rT = nc.dram_tensor("att1r", (P, HC), BF16, kind="ExternalInput")
    Wl2T = nc.dram_tensor("Wl2", (HC, HC), BF16, kind="ExternalInput")
    Wr2T = nc.dram_tensor("Wr2", (HC, HC), BF16, kind="ExternalInput")
    we2rowT = nc.dram_tensor("we2row", (1, HC), BF16, kind="ExternalInput")
    we1rT = nc.dram_tensor("we1r", (P, HC), FP32, kind="ExternalInput")
    we2rT = nc.dram_tensor("we2r", (P, HC), FP32, kind="ExternalInput")
    att2rT = nc.dram_tensor("att2r", (P, HC), BF16, kind="ExternalInput")
    wfcT = nc.dram_tensor("wfc", (P, 52), BF16, kind="ExternalInput")
    bfcrT = nc.dram_tensor("bfcr", (P, NCF), FP32, kind="ExternalInput")
    g1rT = nc.dram_tensor("g1row", (1, HC), FP32, kind="ExternalInput")
    be1rT = nc.dram_tensor("be1row", (1, HC), FP32, kind="ExternalInput")
    g2rT = nc.dram_tensor("g2row", (1, HC), FP32, kind="ExternalInput")
    be2rT = nc.dram_tensor("be2row", (1, HC), FP32, kind="ExternalInput")
    outT = nc.dram_tensor("out", (NPC, NCF), FP32, kind="ExternalOutput")

    # ---- internal DRAM ----
    dbg = bool(os.environ.get("KERNEL_DBG"))
    dbgkind = {"kind": "ExternalOutput"} if dbg else {}
    NXL1 = ((N + P - 1) // P) * P
    hpre1 = nc.dram_tensor("hpre1", (NPC, HC), FP32, **dbgkind)
    h1dbg = (nc.dram_tensor("h1dbg", (NPC, HC), FP32, kind="ExternalOutput")
             if dbg else None)
    xl1 = nc.dram_tensor("xl1", (NXL1, HC), BF16)
    xl2loc = nc.dram_tensor("xl2loc", (NPC, HC), BF16)
    xr2d = nc.dram_tensor("xr2d", (NPC, HC), BF16)
    xl2full = nc.dram_tensor("xl2full", (NPAD, HC), BF16, addr_space="Shared")
    hpre2 = nc.dram_tensor("hpre2", (NPC, HC), FP32, **dbgkind)
    w2i = nc.dram_tensor("w2i", (LE2P, 1), FP32, **dbgkind)
    st1i = nc.dram_tensor("st1i", (1, 2 * HC), FP32)
    st1o = nc.dram_tensor("st1o", (1, 2 * HC), FP32, addr_space="Shared")
    st2i = nc.dram_tensor("st2i", (1, 2 * HC), FP32)
    st2o = nc.dram_tensor("st2o", (1, 2 * HC), FP32, addr_space="Shared")

    with tile.TileContext(nc) as tc:
        with tc.tile_pool(name="cst", bufs=1) as cst, \
             tc.tile_pool(name="sb", bufs=4) as sb, \
             tc.tile_pool(name="sb6", bufs=6) as sb6, \
             tc.tile_pool(name="sb2", bufs=2) as sb2, \
             tc.tile_pool(name="mm", bufs=4, space="PSUM") as mmp, \
             tc.tile_pool(name="tr", bufs=2, space="PSUM") as trp, \
             tc.tile_pool(name="accp", bufs=2, space="PSUM") as accp:

            # ---------------- constants ----------------
            iota_i = cst.tile([P, P], I32, tag="iota_i")
            nc.gpsimd.iota(iota_i[:], pattern=[[1, P]], base=0,
                           channel_multiplier=0)
            iota_f = cst.tile([P, P], FP32, tag="iota_f")
            nc.vector.tensor_copy(iota_f[:], iota_i[:])
            ident32 = cst.tile([P, P], FP32, tag="ident32")
            make_identity(nc, ident32[:])
            identb = cst.tile([P, P], BF16, tag="identb")
            nc.vector.tensor_copy(identb[:], ident32[:])

            def load_const(name, dram, shape, dtype=FP32):
                t = cst.tile(shape, dtype, tag=name)
                nc.sync.dma_start(out=t[:], in_=dram[:, :])
                return t

            Wl1s = load_const("Wl1s", Wl1T, [NCF, HC], BF16)
            we1rs = load_const("we1rs", we1rowT, [1, HC], BF16)
            Wr1s = load_const("Wr1s", Wr1T, [NCF, HC], BF16)
            att1s = load_const("att1s", att1rT, [P, HC], BF16)
            we2rs = load_const("we2rs", we2rowT, [1, HC], BF16)
            we1rep = load_const("we1rep", we1rT, [P, HC])
            we2rep = load_const("we2rep", we2rT, [P, HC])
            att2s = load_const("att2s", att2rT, [P, HC], BF16)
            wfcs = load_const("wfcs", wfcT, [P, 52], BF16)
            bfcs = load_const("bfcs", bfcrT, [P, NCF])
            g1s = load_const("g1s", g1rT, [1, HC])
            be1s = load_const("be1s", be1rT, [1, HC])
            g2s = load_const("g2s", g2rT, [1, HC])
            be2s = load_const("be2s", be2rT, [1, HC])
            Wl2s = cst.tile([P, 2 * HC], BF16, tag="Wl2s")
            Wr2s = cst.tile([P, 2 * HC], BF16, tag="Wr2s")
            for kk in range(2):
                nc.sync.dma_start(out=Wl2s[:, kk * HC:(kk + 1) * HC],
                                  in_=Wl2T[kk * P:(kk + 1) * P, :])
                nc.sync.dma_start(out=Wr2s[:, kk * HC:(kk + 1) * HC],
                                  in_=Wr2T[kk * P:(kk + 1) * P, :])
            ones_col = cst.tile([P, 1], FP32, tag="ones_col")
            nc.vector.memset(ones_col[:], 1.0)
            ones_row = cst.tile([1, P], FP32, tag="ones_row")
            nc.vector.memset(ones_row[:], 1.0)

            st1sb = cst.tile([1, 2 * HC], FP32, tag="st1sb")
            nc.vector.memset(st1sb[:], 0.0)
            st2sb = cst.tile([1, 2 * HC], FP32, tag="st2sb")
            nc.vector.memset(st2sb[:], 0.0)

            # copy host edge weights for layer 2 (self-loop slots get filled
            # by the device during layer-1 finalize)
            nrows = LE2P // P
            for c0 in range(0, nrows, P):
                cn = min(P, nrows - c0)
                w2cp = sb.tile([P, P], FP32, tag="w2cp")
                nc.sync.dma_start(
                    out=w2cp[:cn, :],
                    in_=w2xT[:, 0].rearrange("(a b) -> a b", b=P)[c0:c0 + cn, :])
                nc.sync.dma_start(
                    out=w2i[:, 0].rearrange("(a b) -> a b", b=P)[c0:c0 + cn, :],
                    in_=w2cp[:cn, :])

            # ---------- XL1 = x @ Wl1 for every node (replicated) ----------
            GRP = 4
            nchunks = (N + P - 1) // P          # 391
            for j0 in range(0, nchunks, GRP):
                jn = min(GRP, nchunks - j0)
                c0 = j0 * P
                cn = min(GRP * P, N - c0)
                xtb_sb = sb.tile([NCF, GRP * P], BF16, tag="xtb_sb")
                nc.sync.dma_start(out=xtb_sb[:, 0:cn],
                                  in_=xtbT[:, c0:c0 + cn])
                xl_sb = sb.tile([P, GRP * HC], BF16, tag="xl_sb")
                for j in range(jn):
                    n0 = (j0 + j) * P
                    nn = min(P, N - n0)
                    xl_ps = mmp.tile([P, HC], FP32, tag="mm")
                    nc.tensor.matmul(out=xl_ps[:nn, :],
                                     lhsT=xtb_sb[:, j * P:j * P + nn],
                                     rhs=Wl1s[:], start=True, stop=True)
                    nc.vector.tensor_copy(xl_sb[:nn, j * HC:(j + 1) * HC],
                                          xl_ps[:nn, :])
                full = jn * P if (j0 + jn) * P <= N else None
                if full:
                    nc.sync.dma_start(
                        out=xl1[c0:c0 + jn * P, :].rearrange(
                            "(j p) c -> p j c", p=P),
                        in_=xl_sb[:].rearrange(
                            "p (j c) -> p j c", c=HC)[:, 0:jn, :])
                else:
                    for j in range(jn):
                        n0 = (j0 + j) * P
                        nn = min(P, N - n0)
                        nc.sync.dma_start(out=xl1[n0:n0 + nn, :],
                                          in_=xl_sb[:nn, j * HC:(j + 1) * HC])

            # ================= generic GATv2 edge layer =================
            # Per block:  m = G + XR[dst] + w*We  accumulated in ONE psum
            # group on PE.  The destination-scatter aggregates p*m; the
            # XR/We parts are removed per-tile:
            #   num = S - xr (x) den - We (x) sum(p*w),   h = num / den.
            def edge_layer(layer):
                if layer == 1:
                    NBT, off, eT = NBT1, off1, e1T
                    atts = att1s
                    hpre_dram = hpre1
                    stsb = st1sb
                    ew = 3   # ints per edge record
                    wers, werep = we1rs, we1rep
                    vw = 262
                else:
                    NBT, off, eT = NBT2, off2, e2T
                    atts = att2s
                    hpre_dram = hpre2
                    stsb = st2sb
                    ew = 2
                    wers, werep = we2rs, we2rep
                    vw = 260

                for t in range(NT):
                    nb = NBT[t]
                    # ---- XR tile for the 128 owned nodes ----
                    if layer == 1:
                        xo = sb.tile([P, NCF], BF16, tag="xo")
                        nc.sync.dma_start(out=xo[:],
                                          in_=xownT[t * P:(t + 1) * P, :])
                        xoT_ps = trp.tile([P, P], BF16, tag="tr")
                        nc.tensor.transpose(out=xoT_ps[:NCF, :], in_=xo[:],
                                            identity=identb[:])
                        xoTs = sb.tile([NCF, P], BF16, tag="xoTs")
                        nc.vector.tensor_copy(xoTs[:], xoT_ps[:NCF, :])
                        xr_ps = mmp.tile([P, HC], FP32, tag="mm")
                        nc.tensor.matmul(out=xr_ps[:], lhsT=xoTs[:],
                                         rhs=Wr1s[:], start=True, stop=True)
                        xr_sb = sb.tile([P, HC], BF16, tag="xr_sb")
                        nc.scalar.activation(xr_sb[:], xr_ps[:], AF.Copy)
                    else:
                        xr_sb = sb.tile([P, HC], BF16, tag="xr_sb")
                        nc.sync.dma_start(out=xr_sb[:],
                                          in_=xr2d[t * P:(t + 1) * P, :])

                    # ---- whole tile's edge records in one DMA ----
                    o0 = off[t] * P
                    ebt = sb.tile([P, nb * ew], I32, tag="ebt")
                    nc.sync.dma_start(
                        out=ebt[:].rearrange("p (b c) -> p b c", c=ew),
                        in_=eT[o0:o0 + nb * P, :].rearrange(
                            "(b p) c -> p b c", p=P))
                    wsrc = w1cT if layer == 1 else w2i
                    wrf = sb.tile([1, nb * P], FP32, tag="wrf")
                    nc.sync.dma_start(
                        out=wrf[:],
                        in_=wsrc[o0:o0 + nb * P, :].rearrange("a one -> one a"))
                    wrowt = sb.tile([1, nb * P], BF16, tag="wrowt")
                    nc.vector.tensor_copy(wrowt[:], wrf[:])
                    # per-partition w and float dst for the whole tile
                    wpp = sb.tile([P, nb], FP32, tag="wpp")
                    nc.sync.dma_start(
                        out=wpp[:],
                        in_=wsrc[:, 0].rearrange(
                            "(a p) -> p a", p=P)[:, off[t]:off[t] + nb])
                    d_ft = sb.tile([P, nb], FP32, tag="d_ft")
                    nc.vector.tensor_copy(
                        d_ft[:], ebt[:].rearrange("p (b c) -> p b c", c=ew)[:, :, 1])

                    acc = accp.tile([P, 262], FP32, tag="acc")
                    for b in range(nb):
                        sidx = ebt[:, b * ew:b * ew + 1]

                        # ---- one-hot by local destination ----
                        oh = sb6.tile([P, P], BF16, tag="oh")
                        nc.vector.tensor_scalar(
                            out=oh[:], in0=iota_f[:], scalar1=d_ft[:, b:b + 1],
                            scalar2=None, op0=OP.is_equal)
                        ohT_ps = trp.tile([P, P], BF16, tag="tr")
                        nc.tensor.transpose(out=ohT_ps[:], in_=oh[:],
                                            identity=identb[:])
                        ohTs = sb6.tile([P, P], BF16, tag="ohTs")
                        nc.scalar.activation(ohTs[:], ohT_ps[:], AF.Copy)

                        # ---- gather pre-transformed source rows ----
                        gsrc = xl1 if layer == 1 else xl2full
                        grow = sb2.tile([P, HC], BF16, tag=f"grow{b % 8}")
                        nc.gpsimd.indirect_dma_start(
                            out=grow[:], out_offset=None, in_=gsrc[:, :],
                            in_offset=bass.IndirectOffsetOnAxis(
                                ap=sidx, axis=0))
                        # ---- m = G + XR[dst] + w*We in one psum group ----
                        m_ps = mmp.tile([P, HC], FP32, tag="mm")
                        nc.tensor.matmul(out=m_ps[:], lhsT=identb[:],
                                         rhs=grow[:], start=True, stop=False)
                        nc.tensor.matmul(out=m_ps[:], lhsT=ohTs[:],
                                         rhs=xr_sb[:], start=False, stop=False)
                        nc.tensor.matmul(out=m_ps[:],
                                         lhsT=wrowt[:, b * P:(b + 1) * P],
                                         rhs=wers[:], start=False, stop=True)

                        # ---- attention logits from Prelu(m) ----
                        lrm = sb6.tile([P, HC], BF16, tag="lrm")
                        nc.scalar.activation(lrm[:], m_ps[:], AF.Prelu,
                                             alpha=0.2)
                        junk = sb6.tile([P, P], BF16, tag="junk")
                        alpha = sb6.tile([P, 2], FP32, tag="alpha")
                        for hh in range(2):
                            nc.vector.scalar_tensor_tensor(
                                out=junk[:],
                                in0=lrm[:, hh * CH:(hh + 1) * CH], scalar=1.0,
                                in1=atts[:, hh * CH:(hh + 1) * CH],
                                op0=OP.mult, op1=OP.mult,
                                accum_out=alpha[:, hh:hh + 1])
                        pexp = sb6.tile([P, 2], FP32, tag="pexp")
                        nc.scalar.activation(pexp[:], alpha[:], AF.Exp)

                        # ---- V = [p*m | p | p*w | (w,w)]; scatter by dst ----
                        v = sb6.tile([P, 262], BF16, tag="v")
                        nc.scalar.activation(v[:, 0:CH], m_ps[:, 0:CH],
                                             AF.Copy, scale=pexp[:, 0:1])
                        nc.scalar.activation(v[:, CH:HC], m_ps[:, CH:HC],
                                             AF.Copy, scale=pexp[:, 1:2])
                        nc.vector.tensor_copy(v[:, HC:HC + 2], pexp[:])
                        nc.vector.tensor_scalar_mul(v[:, HC + 2:HC + 4],
                                                    pexp[:], wpp[:, b:b + 1])
                        if layer == 1:
                            nc.vector.tensor_copy(
                                v[:, HC + 4:HC + 6],
                                wpp[:, b:b + 1].to_broadcast([P, 2]))
                        nc.tensor.matmul(out=acc[:, 0:vw], lhsT=oh[:],
                                         rhs=v[:, 0:vw],
                                         start=(b == 0), stop=(b == nb - 1))

                    # ---------------- tile finalize ----------------
                    cp = sb.tile([P, 6], FP32, tag="cp")
                    nc.vector.tensor_copy(cp[:], acc[:, HC:HC + 6])
                    den = sb.tile([P, 2], FP32, tag="den")
                    nc.vector.tensor_scalar_add(den[:], cp[:, 0:2], 1e-16)
                    rden = sb.tile([P, 2], FP32, tag="rden")
                    nc.vector.reciprocal(rden[:], den[:])
                    # ddr = den/(den+eps), swr = swp/(den+eps), both negated
                    nfac = sb.tile([P, 4], FP32, tag="nfac")
                    nc.vector.tensor_mul(nfac[:, 0:2], cp[:, 0:2], rden[:])
                    nc.vector.tensor_mul(nfac[:, 2:4], cp[:, 2:4], rden[:])
                    nc.vector.tensor_scalar_mul(nfac[:], nfac[:], -1.0)
                    hp = sb.tile([P, HC], FP32, tag="hp")
                    c1 = sb.tile([P, HC], FP32, tag="c1")
                    for hh in range(2):
                        cs = slice(hh * CH, (hh + 1) * CH)
                        nc.vector.tensor_scalar_mul(c1[:, cs], acc[:, cs],
                                                    rden[:, hh:hh + 1])
                        nc.vector.scalar_tensor_tensor(
                            out=c1[:, cs], in0=xr_sb[:, cs],
                            scalar=nfac[:, hh:hh + 1], in1=c1[:, cs],
                            op0=OP.mult, op1=OP.add)
                        nc.vector.scalar_tensor_tensor(
                            out=hp[:, cs], in0=werep[:, cs],
                            scalar=nfac[:, 2 + hh:3 + hh], in1=c1[:, cs],
                            op0=OP.mult, op1=OP.add)
                    if layer == 1:
                        ivd = sb.tile([P, 1], FP32, tag="ivd")
                        nc.sync.dma_start(out=ivd[:],
                                          in_=invdT[t * P:(t + 1) * P, :])
                        lat = sb.tile([P, 1], FP32, tag="lat")
                        nc.vector.tensor_mul(lat[:], cp[:, 4:5], ivd[:])
                        slt = sb.tile([P, 1], I32, tag="slt")
                        nc.sync.dma_start(out=slt[:],
                                          in_=slotsT[t * P:(t + 1) * P, :])
                        nc.gpsimd.indirect_dma_start(
                            out=w2i[:, :],
                            out_offset=bass.IndirectOffsetOnAxis(
                                ap=slt[:, :1], axis=0),
                            in_=lat[:], in_offset=None)
                    # stats
                    sq = sb.tile([P, HC], FP32, tag="sq")
                    nc.scalar.activation(sq[:], hp[:], AF.Square)
                    s1_ps = mmp.tile([P, HC], FP32, tag="mm")
                    nc.tensor.matmul(out=s1_ps[0:1, :], lhsT=ones_col[:],
                                     rhs=hp[:], start=True, stop=True)
                    s2_ps = mmp.tile([P, HC], FP32, tag="mm")
                    nc.tensor.matmul(out=s2_ps[0:1, :], lhsT=ones_col[:],
                                     rhs=sq[:], start=True, stop=True)
                    nc.vector.tensor_add(stsb[0:1, 0:HC], stsb[0:1, 0:HC],
                                         s1_ps[0:1, :])
                    nc.vector.tensor_add(stsb[0:1, HC:2 * HC],
                                         stsb[0:1, HC:2 * HC], s2_ps[0:1, :])
                    nc.sync.dma_start(out=hpre_dram[t * P:(t + 1) * P, :],
                                      in_=hp[:])

            # ============ BN finalize: AllReduce stats + scale/shift ============
            def bn_scale_shift(stsb, sti, sto, grow_s, brow_s):
                nc.sync.dma_start(out=sti[:, :], in_=stsb[:])
                nc.gpsimd.collective_compute(
                    "AllReduce", OP.add, ins=[sti[:, :]], outs=[sto[:, :]],
                    replica_groups=RG)
                stg = sb.tile([1, 2 * HC], FP32, tag="stg")
                nc.sync.dma_start(out=stg[:], in_=sto[:, :])
                mu = sb.tile([1, HC], FP32, tag="mu")
                nc.vector.tensor_scalar_mul(mu[:], stg[0:1, 0:HC], 1.0 / N)
                msq = sb.tile([1, HC], FP32, tag="msq")
                nc.vector.tensor_scalar_mul(msq[:], stg[0:1, HC:2 * HC], 1.0 / N)
                musq = sb.tile([1, HC], FP32, tag="musq")
                nc.vector.tensor_mul(musq[:], mu[:], mu[:])
                var = sb.tile([1, HC], FP32, tag="var")
                nc.vector.tensor_sub(var[:], msq[:], musq[:])
                nc.vector.tensor_scalar_add(var[:], var[:], 1e-5)
                sd = sb.tile([1, HC], FP32, tag="sd")
                nc.scalar.activation(sd[:], var[:], AF.Sqrt)
                rsd = sb.tile([1, HC], FP32, tag="rsd")
                nc.vector.reciprocal(rsd[:], sd[:])
                scl = sb.tile([1, HC], FP32, tag="scl")
                nc.vector.tensor_mul(scl[:], grow_s[:], rsd[:])
                sclmu = sb.tile([1, HC], FP32, tag="sclmu")
                nc.vector.tensor_mul(sclmu[:], scl[:], mu[:])
                shf = sb.tile([1, HC], FP32, tag="shf")
                nc.vector.tensor_sub(shf[:], brow_s[:], sclmu[:])
                scl_ps = mmp.tile([P, HC], FP32, tag="mm")
                nc.tensor.matmul(out=scl_ps[:], lhsT=ones_row[:], rhs=scl[:],
                                 start=True, stop=True)
                sclb = cst.tile([P, HC], FP32, tag=f"sclb{id(stsb)}")
                nc.scalar.activation(sclb[:], scl_ps[:], AF.Copy)
                shf_ps = mmp.tile([P, HC], FP32, tag="mm")
                nc.tensor.matmul(out=shf_ps[:], lhsT=ones_row[:], rhs=shf[:],
                                 start=True, stop=True)
                shfb = cst.tile([P, HC], FP32, tag=f"shfb{id(stsb)}")
                nc.scalar.activation(shfb[:], shf_ps[:], AF.Copy)
                return sclb, shfb

            # ================= run the whole network =================
            edge_layer(1)
            scl1b, shf1b = bn_scale_shift(st1sb, st1i, st1o, g1s, be1s)

            # BN1 apply + leaky relu + dropout mask -> h1act (bf16),
            # fused with the layer-2 XR precompute (overlaps the AllGather)
            for t in range(NT):
                ht = sb.tile([P, HC], FP32, tag="ht")
                nc.sync.dma_start(out=ht[:], in_=hpre1[t * P:(t + 1) * P, :])
                t1 = sb.tile([P, HC], FP32, tag="t1")
                nc.vector.tensor_mul(t1[:], ht[:], scl1b[:])
                nc.vector.tensor_add(t1[:], t1[:], shf1b[:])
                ha = sb.tile([P, HC], FP32, tag="ha")
                nc.scalar.activation(ha[:], t1[:], AF.Prelu, alpha=0.01)
                mk = sb.tile([P, HC], FP32, tag="mk")
                nc.sync.dma_start(out=mk[:], in_=mask1T[t * P:(t + 1) * P, :])
                hm = sb.tile([P, HC], BF16, tag="hm")
                nc.vector.tensor_mul(hm[:], ha[:], mk[:])
                xoTs = sb.tile([P, HC], BF16, tag="xoTs2")
                for kk in range(2):
                    tp = trp.tile([P, P], BF16, tag="tr")
                    nc.tensor.transpose(out=tp[:],
                                        in_=hm[:, kk * P:(kk + 1) * P],
                                        identity=identb[:])
                    nc.scalar.activation(xoTs[:, kk * P:(kk + 1) * P], tp[:],
                                         AF.Copy)
                xr_ps = mmp.tile([P, HC], FP32, tag="mm")
                for kk in range(2):
                    nc.tensor.matmul(out=xr_ps[:],
                                     lhsT=xoTs[:, kk * P:(kk + 1) * P],
                                     rhs=Wr2s[:, kk * HC:(kk + 1) * HC],
                                     start=(kk == 0), stop=(kk == 1))
                xr2_t = sb.tile([P, HC], BF16, tag="xr2_t")
                nc.scalar.activation(xr2_t[:], xr_ps[:], AF.Copy)
                nc.sync.dma_start(out=xr2d[t * P:(t + 1) * P, :], in_=xr2_t[:])
                xl2_ps = mmp.tile([P, HC], FP32, tag="mm")
                for kk in range(2):
                    nc.tensor.matmul(out=xl2_ps[:],
                                     lhsT=xoTs[:, kk * P:(kk + 1) * P],
                                     rhs=Wl2s[:, kk * HC:(kk + 1) * HC],
                                     start=(kk == 0), stop=(kk == 1))
                xl2_sb = sb.tile([P, HC], BF16, tag="xl2_sb")
                nc.scalar.activation(xl2_sb[:], xl2_ps[:], AF.Copy)
                nc.sync.dma_start(out=xl2loc[t * P:(t + 1) * P, :], in_=xl2_sb[:])
                if h1dbg is not None:
                    hmf = sb.tile([P, HC], FP32, tag="hmf")
                    nc.vector.tensor_copy(hmf[:], hm[:])
                    nc.sync.dma_start(out=h1dbg[t * P:(t + 1) * P, :], in_=hmf[:])

            # all-gather the transformed layer-2 source features (bf16)
            nc.gpsimd.collective_compute(
                "AllGather", OP.bypass, ins=[xl2loc[:, :]], outs=[xl2full[:, :]],
                replica_groups=RG)

            edge_layer(2)
            scl2b, shf2b = bn_scale_shift(st2sb, st2i, st2o, g2s, be2s)

            # BN2 apply + leaky relu + dropout + FC
            for t in range(NT):
                ht = sb.tile([P, HC], FP32, tag="ht2")
                nc.sync.dma_start(out=ht[:], in_=hpre2[t * P:(t + 1) * P, :])
                t1 = sb.tile([P, HC], FP32, tag="t12")
                nc.vector.tensor_mul(t1[:], ht[:], scl2b[:])
                nc.vector.tensor_add(t1[:], t1[:], shf2b[:])
                ha = sb.tile([P, HC], FP32, tag="ha2")
                nc.scalar.activation(ha[:], t1[:], AF.Prelu, alpha=0.01)
                mk = sb.tile([P, HC], FP32, tag="mk2")
                nc.sync.dma_start(out=mk[:], in_=mask2T[t * P:(t + 1) * P, :])
                hm = sb.tile([P, HC], BF16, tag="hm2")
                nc.vector.tensor_mul(hm[:], ha[:], mk[:])
                h2T = sb.tile([P, HC], BF16, tag="h2T")
                for kk in range(2):
                    tp = trp.tile([P, P], BF16, tag="tr")
                    nc.tensor.transpose(out=tp[:],
                                        in_=hm[:, kk * P:(kk + 1) * P],
                                        identity=identb[:])
                    nc.vector.tensor_copy(h2T[:, kk * P:(kk + 1) * P], tp[:])
                fc_ps = mmp.tile([P, HC], FP32, tag="mm")
                for kk in range(2):
                    nc.tensor.matmul(out=fc_ps[:, 0:NCF],
                                     lhsT=h2T[:, kk * P:(kk + 1) * P],
                                     rhs=wfcs[:, kk * NCF:(kk + 1) * NCF],
                                     start=(kk == 0), stop=(kk == 1))
                ob = sb.tile([P, NCF], FP32, tag="ob")
                nc.vector.tensor_add(ob[:], fc_ps[:, 0:NCF], bfcs[:])
                nc.sync.dma_start(out=outT[t * P:(t + 1) * P, :], in_=ob[:])

    nc.compile()
    return nc


# ----------------------------------------------------------------------------
# entry point
# ----------------------------------------------------------------------------

def kernel(x_input, edge_weight, params, edge_index):
    in_maps, meta = _host_prep(x_input, edge_weight, params, edge_index)
    key = tuple(sorted(meta.items()))
    if key not in _PROGRAM_CACHE:
        _PROGRAM_CACHE[key] = _build_program(meta)
    nc = _PROGRAM_CACHE[key]
    res = run_bass_kernel_spmd(nc, in_maps, core_ids=list(range(NCORES)))
    if os.environ.get("KERNEL_DBG"):
        kernel.last_res = res
        kernel.last_meta = meta
    if res.exec_time_ns is not None:
        print(f"HW exec time: {res.exec_time_ns} ns")
    out = np.empty((N, NCF), np.float32)
    for r in range(NCORES):
        out[r * NOWN:(r + 1) * NOWN] = res.results[r]["out"][:NOWN]
    return out
